# revision 1
# baseline (speedup 1.0000x reference)
"""Cross-attention kernel for TRN2, 8 NeuronCores.

Sharding: core c -> (batch b = c//2, head-group g = c%2).  Each head-group is
8 heads = 512 of the 1024 d_model channels.  Within a core:
  QT = (Wq_g/8) @ q_b.T + bq_g/8        [512, 512]   (s, lq)   scale folded
  KT = Wk_g @ kv_b.T + bk_g             [512, 2048]  (s, lkv)
  V  = kv_b @ Wv_g.T + bv_g             [2048, 512]  (lkv, s)
  ST_h = Kh @ Qh.T                      [2048, 512]  per head (lkv, lq)
  PT_h = exp(ST_h)        (no max-sub: scores ~N(0,1), bounded)
  cT_h = V_h.T @ PT_h / colsum(PT_h)    [64, 512]    (s, lq)
  out_partial = cT.T.T @ Wo_g.T         [512, 1024]  (lq, d)
Host sums the two head-group partials per batch and adds bo.

All matmuls run as float32r (TF32-ish, full PE rate at N=512).  Head pairs are
packed onto the 128-wide PE array via partition-offset row/col tiling.
"""

import sys
if "/opt/trn_rl_repo" not in sys.path:
    sys.path.insert(0, "/opt/trn_rl_repo")

import numpy as np

import concourse.bass as bass
import concourse.mybir as mybir
import concourse.tile as tile
from concourse.bass_utils import run_bass_kernel_spmd

f32 = mybir.dt.float32
f32r = mybir.dt.float32r
EXP = mybir.ActivationFunctionType.Exp
IDENT = mybir.ActivationFunctionType.Identity

D = 1024        # d_model
S = 512         # per-core channel shard (8 heads x 64)
LQ = 512
LKV = 2048
CO = D // 128   # 8 contraction chunks
SO = S // 128   # 4 shard s-tiles
NT = LKV // 128  # 16 lkv tiles
NKC = LKV // 512  # 4 lkv 512-chunks


def _split_multi_waits(nc, max_waits=1):
    """This container's walrus allows only `max_waits` sync-wait commands per
    instruction; hoist the excess into standalone EventSemaphore insts."""
    ev_id = 0
    for f in nc.m.functions:
        for bb in f.blocks:
            new = []
            changed = False
            for inst in bb.instructions:
                si = inst.sync_info
                if si is not None and si.on_wait and len(si.on_wait) > max_waits:
                    waits = list(si.on_wait)
                    for sw in waits[:-max_waits]:
                        ev = mybir.InstEventSemaphore(
                            name=f"EVSPLIT-{ev_id}", engine=inst.engine,
                            sync_info=mybir.SyncInfo(on_wait=[sw], on_update=[]))
                        ev_id += 1
                        nc.register_instruction(ev, overwrite=True)
                        new.append(ev)
                    inst.sync_info = mybir.SyncInfo(
                        on_wait=waits[-max_waits:], on_update=list(si.on_update))
                    changed = True
                new.append(inst)
            if changed:
                bb.instructions = new
    return nc


def _build():
    nc = bass.Bass(trn_type="TRN2")

    # DRAM I/O (activations/weights pre-laid-out [128, outer, free] on host)
    qT = nc.dram_tensor("qT", [128, CO, LQ], f32r, kind="ExternalInput")
    kvT = nc.dram_tensor("kvT", [128, CO, LKV], f32r, kind="ExternalInput")
    wqT = nc.dram_tensor("wqT", [128, CO, S], f32r, kind="ExternalInput")
    wkT = nc.dram_tensor("wkT", [128, CO, S], f32r, kind="ExternalInput")
    wvT = nc.dram_tensor("wvT", [128, CO, S], f32r, kind="ExternalInput")
    woT = nc.dram_tensor("woT", [128, SO, D], f32r, kind="ExternalInput")
    bq = nc.dram_tensor("bq", [128, SO], f32, kind="ExternalInput")
    bk = nc.dram_tensor("bk", [128, SO], f32, kind="ExternalInput")
    bv = nc.dram_tensor("bv", [1, S], f32r, kind="ExternalInput")
    out = nc.dram_tensor("out", [SO, 128, D], f32, kind="ExternalOutput")

    with tile.TileContext(nc) as tc:
        with tc.tile_pool(name="wgt", bufs=1) as wgt, \
             tc.tile_pool(name="big", bufs=1) as big, \
             tc.tile_pool(name="strm", bufs=3) as strm, \
             tc.tile_pool(name="pt", bufs=2) as ptp, \
             tc.tile_pool(name="ostg", bufs=2) as ostg, \
             tc.tile_pool(name="sml", bufs=2) as sml, \
             tc.tile_pool(name="psA", bufs=4, space="PSUM") as psA, \
             tc.tile_pool(name="psC", bufs=2, space="PSUM") as psC:

            # ---- resident weights / constants ----
            wk_sb = wgt.tile([128, CO, S], f32r, name="wk_sb")
            wv_sb = wgt.tile([128, CO, S], f32r, name="wv_sb")
            kv_sb = wgt.tile([128, CO, LKV], f32r, name="kv_sb")
            bq_sb = wgt.tile([128, SO], f32, name="bq_sb")
            bk_sb = wgt.tile([128, SO], f32, name="bk_sb")
            bv_sb = wgt.tile([1, S], f32r, name="bv_sb")
            ones_f = wgt.tile([128, 128], f32, name="ones_f")
            ones = wgt.tile([128, 128], f32r, name="ones")
            for c in range(CO):
                nc.sync.dma_start(wk_sb[:, c, :], wkT[:, c, :])
                nc.sync.dma_start(wv_sb[:, c, :], wvT[:, c, :])
                nc.sync.dma_start(kv_sb[:, c, :], kvT[:, c, :])
            nc.sync.dma_start(bq_sb, bq[:])
            nc.sync.dma_start(bk_sb, bk[:])
            nc.sync.dma_start(bv_sb, bv[:])
            nc.vector.memset(ones_f, 1.0)
            nc.vector.tensor_copy(ones, ones_f)

            # broadcast bv across partitions: ones[1,128].T @ bv[1,512]
            bv_ps = psA.tile([128, S], f32, name="bv_ps", tag="mm")
            nc.tensor.matmul(bv_ps, ones[0:1, :], bv_sb, start=True, stop=True)
            bv_bc = wgt.tile([128, S], f32r, name="bv_bc")
            nc.vector.tensor_copy(bv_bc, bv_ps)

            # ---- resident intermediates ----
            KT_sb = big.tile([128, SO, LKV], f32r, name="KT_sb")   # (s, lkv)
            # V padded per head with a ones column: [lkv, t, head, 64+1].
            # The ones column makes attn@V also produce the softmax
            # denominator as psum row 64 (col-tiling to upper partitions is
            # rejected by this walrus, so no separate denominator matmuls).
            Vp_sb = big.tile([128, NT, 8, 65], f32r, name="Vp_sb")
            QT_sb = big.tile([128, SO, LQ], f32r, name="QT_sb")    # (s, lq)
            cT_sb = big.tile([128, SO, LQ], f32r, name="cT_sb")    # (s, lq)
            nc.vector.tensor_copy(
                Vp_sb[:, :, :, 64:65],
                ones_f[:, 0:128].rearrange("p (a b c) -> p a b c", a=NT, b=8, c=1))

            # ---- K projection: KT[s, lkv] += wk[c,s].T @ kv_sb[c, lkv] ----
            for ch in range(NKC):
                kps = [psA.tile([128, 512], f32, name=f"kps{o}_{ch}", tag="mm")
                       for o in range(SO)]
                for c in range(CO):
                    for o in range(SO):
                        nc.tensor.matmul(
                            kps[o], wk_sb[:, c, o * 128:(o + 1) * 128],
                            kv_sb[:, c, ch * 512:(ch + 1) * 512],
                            start=(c == 0), stop=(c == CO - 1))
                for o in range(SO):
                    nc.scalar.activation(
                        KT_sb[:, o, ch * 512:(ch + 1) * 512], kps[o], IDENT,
                        bias=bk_sb[:, o:o + 1])

            # ---- V projection: V[lkv, s] += kv_sb[c, lkv].T @ wv[c, s] ----
            for t in range(NT):
                vps = psA.tile([128, 512], f32, name="vps", tag="mm")
                for c in range(CO):
                    nc.tensor.matmul(vps, kv_sb[:, c, t * 128:(t + 1) * 128],
                                     wv_sb[:, c, :],
                                     start=(c == 0), stop=(c == CO - 1))
                nc.vector.tensor_add(
                    Vp_sb[:, t, :, 0:64],
                    vps.rearrange("p (h d) -> p h d", h=8),
                    bv_bc.rearrange("p (h d) -> p h d", h=8))

            # ---- Q projection: QT[s, lq] += wq[c,s].T @ qT[c, lq] ----
            qps = [psA.tile([128, 512], f32, name=f"qps{o}", tag="mm")
                   for o in range(SO)]
            for c in range(CO):
                qtt = strm.tile([128, 512], f32r, name="qtt", tag="st512")
                nc.sync.dma_start(qtt, qT[:, c, :])
                wqc = strm.tile([128, S], f32r, name="wqc", tag="st512")
                nc.sync.dma_start(wqc, wqT[:, c, :])
                for o in range(SO):
                    nc.tensor.matmul(
                        qps[o], wqc[:, o * 128:(o + 1) * 128], qtt,
                        start=(c == 0), stop=(c == CO - 1))
            for o in range(SO):
                nc.scalar.activation(QT_sb[:, o, :], qps[o], IDENT,
                                     bias=bq_sb[:, o:o + 1])

            # ---- attention, head pairs (2o, 2o+1) ----
            # scores row-tiled (dh=64 contraction at row offsets 0/64);
            # attn@V per head with M=65 (64 V cols + ones col -> denominator
            # lands in psum row 64).
            for o in range(SO):
                ctxA = psC.tile([65, 512], f32, name="ctxA", tag="ctxA")
                ctxB = psC.tile([65, 512], f32, name="ctxB", tag="ctxB")
                for t in range(NT):
                    stA = psA.tile([128, 512], f32, name="stA", tag="mm")
                    stB = psA.tile([128, 512], f32, name="stB", tag="mm")
                    # S.T tile = Kh[., t-slice].T-contraction over dh=64 rows
                    nc.tensor.matmul(stA, KT_sb[0:64, o, t * 128:(t + 1) * 128],
                                     QT_sb[0:64, o, :], start=True, stop=True)
                    nc.tensor.matmul(stB, KT_sb[64:128, o, t * 128:(t + 1) * 128],
                                     QT_sb[64:128, o, :], start=True, stop=True)
                    ptA = ptp.tile([128, 512], f32r, name="ptA", tag="ptA")
                    ptB = ptp.tile([128, 512], f32r, name="ptB", tag="ptB")
                    nc.scalar.activation(ptA, stA, EXP)
                    nc.scalar.activation(ptB, stB, EXP)
                    st = (t == 0)
                    sp = (t == NT - 1)
                    nc.tensor.matmul(ctxA, Vp_sb[:, t, 2 * o, :], ptA,
                                     start=st, stop=sp)
                    nc.tensor.matmul(ctxB, Vp_sb[:, t, 2 * o + 1, :], ptB,
                                     start=st, stop=sp)
                # normalize: cT_h = ctx_h[0:64] * (1/ctx_h[64]) bcast to 64 rows
                for h, ctx in ((0, ctxA), (1, ctxB)):
                    rc = sml.tile([1, 512], f32r, name="rc", tag="rc")
                    with nc.allow_low_precision(reason="softmax recip f32r"):
                        nc.vector.reciprocal(rc, ctx[64:65, :])
                    nb_ps = psA.tile([64, 512], f32, name="nb_ps", tag="mm")
                    nc.tensor.matmul(nb_ps, ones[0:1, 0:64], rc,
                                     start=True, stop=True)
                    nb_sb = sml.tile([64, 512], f32, name="nb_sb", tag="nb")
                    nc.vector.tensor_copy(nb_sb, nb_ps)
                    nc.vector.tensor_mul(
                        cT_sb[h * 64:(h + 1) * 64, o, :], ctx[0:64, :], nb_sb)

            # ---- out projection: out[lq, d] += cT[s, lq-slice].T @ wo[s, d] ----
            for dc in range(2):
                opss = [psA.tile([128, 512], f32, name=f"ops{lt}", tag="mm")
                        for lt in range(SO)]
                for o in range(SO):
                    woc = strm.tile([128, 512], f32r, name="woc", tag="st512")
                    nc.sync.dma_start(woc, woT[:, o, dc * 512:(dc + 1) * 512])
                    for lt in range(SO):
                        nc.tensor.matmul(
                            opss[lt], cT_sb[:, o, lt * 128:(lt + 1) * 128],
                            woc, start=(o == 0), stop=(o == SO - 1))
                for lt in range(SO):
                    ot = ostg.tile([128, 512], f32, name="ot", tag="ot")
                    nc.vector.tensor_copy(ot, opss[lt])
                    nc.sync.dma_start(out[lt, :, dc * 512:(dc + 1) * 512], ot)

    return _split_multi_waits(nc)


_NC = None


def _get_nc():
    global _NC
    if _NC is None:
        _NC = _build()
    return _NC


def _shard(q, kv, Wq, bq, Wk, bk, Wv, bv, Wo, bo):
    def lay(a2d, co):  # [co*128, F] -> [128, co, F]
        F = a2d.shape[1]
        return np.ascontiguousarray(
            a2d.reshape(co, 128, F).transpose(1, 0, 2))

    in_maps = []
    for core in range(8):
        b, g = core // 2, core % 2
        sl = slice(g * S, (g + 1) * S)
        m = {
            "qT": lay(np.ascontiguousarray(q[b].T), CO),
            "kvT": lay(np.ascontiguousarray(kv[b].T), CO),
            "wqT": lay(np.ascontiguousarray((Wq[sl] * 0.125).T), CO),
            "wkT": lay(np.ascontiguousarray(Wk[sl].T), CO),
            "wvT": lay(np.ascontiguousarray(Wv[sl].T), CO),
            "woT": lay(np.ascontiguousarray(Wo[:, sl].T), SO),
            "bq": np.ascontiguousarray((bq[sl] * 0.125).reshape(SO, 128).T),
            "bk": np.ascontiguousarray(bk[sl].reshape(SO, 128).T),
            "bv": np.ascontiguousarray(bv[sl].reshape(1, S)),
        }
        in_maps.append({k: v.astype(np.float32, copy=False) for k, v in m.items()})
    return in_maps


def _run(in_maps, trace=False):
    res = run_bass_kernel_spmd(_get_nc(), in_maps, core_ids=list(range(8)),
                               trace=trace)
    return res


def kernel(q, kv, Wq, bq, Wk, bk, Wv, bv, Wo, bo, _trace=False):
    q, kv = np.asarray(q, np.float32), np.asarray(kv, np.float32)
    Wq, Wk = np.asarray(Wq, np.float32), np.asarray(Wk, np.float32)
    Wv, Wo = np.asarray(Wv, np.float32), np.asarray(Wo, np.float32)
    bq, bk = np.asarray(bq, np.float32), np.asarray(bk, np.float32)
    bv, bo = np.asarray(bv, np.float32), np.asarray(bo, np.float32)

    in_maps = _shard(q, kv, Wq, bq, Wk, bk, Wv, bv, Wo, bo)
    res = _run(in_maps, trace=_trace)
    B = q.shape[0]
    outp = np.empty((B, LQ, D), np.float32)
    for b in range(B):
        p0 = res.results[2 * b]["out"].reshape(LQ, D)
        p1 = res.results[2 * b + 1]["out"].reshape(LQ, D)
        outp[b] = p0 + p1 + bo[None, :]
    if _trace:
        kernel._last_exec_ns = res.exec_time_ns
        kernel._last_trace = res.instructions_and_trace
    return outp



# revision 7
# speedup vs baseline: 1.7190x; 1.7190x over previous
"""Cross-attention kernel for TRN2, 8 NeuronCores.

Sharding: core c -> (batch b = c//2, head-group g = c%2).  Each head-group is
8 heads = 512 of the 1024 d_model channels.  Within a core everything runs in
one fused software pipeline over 4 head-pair stages (o = 0..3):

  QT = wq_g.T @ q.T  (scale folded)       [512, 512]   (s, lq)
  KT = wk_g.T @ kv.T                      [512, 2048]  (s, lkv)
  V  = kv @ wv_g.T                        [2048, 512]  (lkv, s)   + ones col
  phase1(o), t = 0..15:
               ST = Kh.T-contract @ QT    [128, 512]   (lkv-tile, lq)
               P[t] = exp(ST)          -> bf16 SBUF [128, 16, 512] per head
  phase2(o), unit (hp, lt):  16 consecutive matmuls in ONE psum bank
               ctx[lq, 65] += P[t]_lt.T @ [Vh | 1]     (F=65 transposed form;
                                                        col 64 = softmax denom)
               C = ctx[:, 0:64] * recip(ctx[:, 64])  (DVE per-partition scalar)
  transpose C -> cT[s, lq]  (PE transpose, identity trick)
  out = cT.T @ wo_g.T                     [512, 1024]
Host sums the two head-group partials per batch and adds bo.

All operands are bf16 (1 cyc/row on PE at any free size; halves DMA), psum
accumulation f32.  phase2(o-1) and the projection matmuls of stage o+1 are
hand-interleaved into phase1(o)'s t-loop so the Act engine's exp stream (the
second largest engine load) fully overlaps PE work.  A psum accumulation
group owns its whole 2KB bank (start zeroes the full zero-region), hence the
consecutive-16 structure of phase2 rather than round-robin accumulation.
"""

import sys
if "/opt/trn_rl_repo" not in sys.path:
    sys.path.insert(0, "/opt/trn_rl_repo")

import numpy as np
import ml_dtypes

import concourse.bass as bass
import concourse.mybir as mybir
import concourse.tile as tile
from concourse.bass_utils import run_bass_kernel_spmd

f32 = mybir.dt.float32
bf16 = mybir.dt.bfloat16
EXP = mybir.ActivationFunctionType.Exp
IDENT = mybir.ActivationFunctionType.Identity

D = 1024        # d_model
S = 512         # per-core channel shard (8 heads x 64)
LQ = 512
LKV = 2048
CO = D // 128   # 8 contraction chunks
SO = S // 128   # 4 shard s-tiles (head pairs)
NT = LKV // 128  # 16 lkv tiles


def _split_multi_waits(nc, max_waits=1):
    """This container's walrus allows only `max_waits` sync-wait commands per
    instruction; hoist the excess into standalone EventSemaphore insts."""
    ev_id = 0
    for f in nc.m.functions:
        for bb in f.blocks:
            new = []
            changed = False
            for inst in bb.instructions:
                si = inst.sync_info
                if si is not None and si.on_wait and len(si.on_wait) > max_waits:
                    waits = list(si.on_wait)
                    for sw in waits[:-max_waits]:
                        ev = mybir.InstEventSemaphore(
                            name=f"EVSPLIT-{ev_id}", engine=inst.engine,
                            sync_info=mybir.SyncInfo(on_wait=[sw], on_update=[]))
                        ev_id += 1
                        nc.register_instruction(ev, overwrite=True)
                        new.append(ev)
                    inst.sync_info = mybir.SyncInfo(
                        on_wait=waits[-max_waits:], on_update=list(si.on_update))
                    changed = True
                new.append(inst)
            if changed:
                bb.instructions = new
    return nc


def _build():
    nc = bass.Bass(trn_type="TRN2")

    # DRAM I/O (pre-laid-out [128, outer, free] on host, bf16)
    qT = nc.dram_tensor("qT", [128, CO, LQ], bf16, kind="ExternalInput")
    kvT = nc.dram_tensor("kvT", [128, CO, LKV], bf16, kind="ExternalInput")
    wqT = nc.dram_tensor("wqT", [128, CO, S], bf16, kind="ExternalInput")
    wkT = nc.dram_tensor("wkT", [128, CO, S], bf16, kind="ExternalInput")
    wvT = nc.dram_tensor("wvT", [128, CO, S], bf16, kind="ExternalInput")
    woT = nc.dram_tensor("woT", [128, SO, D], bf16, kind="ExternalInput")
    bq = nc.dram_tensor("bq", [128, SO], f32, kind="ExternalInput")
    bk = nc.dram_tensor("bk", [128, SO], f32, kind="ExternalInput")
    bvbc = nc.dram_tensor("bvbc", [128, S], bf16, kind="ExternalInput")
    ident = nc.dram_tensor("ident", [128, 128], bf16, kind="ExternalInput")
    out = nc.dram_tensor("out", [SO, 128, D], f32, kind="ExternalOutput")

    with tile.TileContext(nc) as tc:
        with tc.tile_pool(name="wgt", bufs=1) as wgt, \
             tc.tile_pool(name="pt", bufs=2) as ptp, \
             tc.tile_pool(name="stg", bufs=4) as stg, \
             tc.tile_pool(name="ost", bufs=3) as ost, \
             tc.tile_pool(name="ps", bufs=1, space="PSUM") as ps:

            # ---- resident SBUF ----
            kv_sb = wgt.tile([128, CO, LKV], bf16, name="kv_sb")
            wk_sb = wgt.tile([128, CO, S], bf16, name="wk_sb")
            wv_sb = wgt.tile([128, CO, S], bf16, name="wv_sb")
            wq_sb = wgt.tile([128, CO, S], bf16, name="wq_sb")
            wo_sb = wgt.tile([128, SO, D], bf16, name="wo_sb")
            qT_sb = wgt.tile([128, CO, LQ], bf16, name="qT_sb")
            QT_sb = wgt.tile([128, SO, LQ], bf16, name="QT_sb")
            KT_sb = wgt.tile([128, SO, LKV], bf16, name="KT_sb")
            # V per head with a ones column: attn@V (transposed form) then
            # also yields the softmax denominator in output col 64.
            Vp_sb = wgt.tile([128, NT, 8, 65], bf16, name="Vp_sb")
            cT_sb = wgt.tile([128, SO, LQ], bf16, name="cT_sb")
            bq_sb = wgt.tile([128, SO], f32, name="bq_sb")
            bk_sb = wgt.tile([128, SO], f32, name="bk_sb")
            bvbc_sb = wgt.tile([128, S], bf16, name="bvbc_sb")
            ident_sb = wgt.tile([128, 128], bf16, name="ident_sb")

            # ---- DMA order = priority order (SP queue is serial) ----
            nc.sync.dma_start(bq_sb, bq[:])
            nc.sync.dma_start(bk_sb, bk[:])
            nc.sync.dma_start(bvbc_sb, bvbc[:])
            nc.sync.dma_start(ident_sb, ident[:])
            nc.sync.dma_start(wk_sb, wkT[:])
            nc.sync.dma_start(kv_sb[:, :, 0:512], kvT[:, :, 0:512])
            nc.sync.dma_start(wq_sb, wqT[:])
            nc.sync.dma_start(qT_sb, qT[:])
            nc.sync.dma_start(wv_sb, wvT[:])
            for ch in range(1, 4):
                nc.sync.dma_start(kv_sb[:, :, ch * 512:(ch + 1) * 512],
                                  kvT[:, :, ch * 512:(ch + 1) * 512])
            nc.sync.dma_start(wo_sb, woT[:])

            nc.vector.memset(Vp_sb[:, :, :, 64:65], 1.0)

            # ---- emission helpers (each emits PE matmuls + its drain) ----
            def kproj(o, ch):
                kps = ps.tile([128, 512], f32, name=f"kps{o}_{ch}", tag="proj",
                              bufs=2)
                sl = slice(ch * 512, (ch + 1) * 512)
                for c in range(CO):
                    nc.tensor.matmul(kps, wk_sb[:, c, o * 128:(o + 1) * 128],
                                     kv_sb[:, c, sl],
                                     start=(c == 0), stop=(c == CO - 1))
                nc.vector.tensor_scalar_add(KT_sb[:, o, sl], kps,
                                            bk_sb[:, o:o + 1])

            def qproj(o):
                qps = ps.tile([128, 512], f32, name=f"qps{o}", tag="proj",
                              bufs=2)
                for c in range(CO):
                    nc.tensor.matmul(qps, wq_sb[:, c, o * 128:(o + 1) * 128],
                                     qT_sb[:, c, :],
                                     start=(c == 0), stop=(c == CO - 1))
                nc.vector.tensor_scalar_add(QT_sb[:, o, :], qps,
                                            bq_sb[:, o:o + 1])

            def vproj(o, t):
                vps = ps.tile([128, 128], f32, name=f"vps{o}_{t}", tag="proj",
                              bufs=2)
                tsl = slice(t * 128, (t + 1) * 128)
                osl = slice(o * 128, (o + 1) * 128)
                for c in range(CO):
                    nc.tensor.matmul(vps, kv_sb[:, c, tsl], wv_sb[:, c, osl],
                                     start=(c == 0), stop=(c == CO - 1))
                nc.vector.tensor_add(
                    Vp_sb[:, t, 2 * o:2 * o + 2, 0:64],
                    vps.rearrange("p (h d) -> p h d", h=2),
                    bvbc_sb[:, osl].rearrange("p (h d) -> p h d", h=2))

            # ---- lead-in: stage-0 prerequisites ----
            kproj(0, 0)
            qproj(0)

            # deferred fill units, emitted round-robin inside stages
            def mk_fill(o):
                f = []
                if o == 0:
                    for ch in range(1, 4):
                        f.append(lambda ch=ch: kproj(0, ch))
                if o < 3:
                    f.append(lambda o=o: qproj(o + 1))
                    for ch in range(4):
                        f.append(lambda o=o, ch=ch: kproj(o + 1, ch))
                return f

            def phase2_unit(o, pts, hp, lt, c_sb):
                """ctx unit (head hp of pair o, lq tile lt): 16 consecutive
                matmuls in one psum bank, then normalize straight from psum.
                (An accumulation group owns its whole 2KB zero-region, so the
                16 steps must be consecutive in one dedicated bank.)"""
                pt = pts[hp]
                ctx = ps.tile([128, 65], f32, name=f"ctx{o}_{hp}_{lt}",
                              tag="ctx", bufs=2)
                for t in range(NT):
                    nc.tensor.matmul(
                        ctx, pt[:, t, lt * 128:(lt + 1) * 128],
                        Vp_sb[:, t, 2 * o + hp, :],
                        start=(t == 0), stop=(t == NT - 1))
                rc = stg.tile([128, 1], f32, name=f"rc{o}_{hp}_{lt}", tag="rc",
                              bufs=4)
                nc.vector.reciprocal(rc, ctx[:, 64:65])
                nc.vector.tensor_scalar_mul(
                    c_sb[:, hp, lt, :], ctx[:, 0:64], rc)

            def transpose_pair(o, hp, c_sb):
                trp = ps.tile([128, SO, 128], bf16, name=f"trp{o}_{hp}",
                              tag="proj", bufs=2)
                for lt in range(SO):
                    nc.tensor.transpose(trp[0:64, lt, :],
                                        c_sb[:, hp, lt, :], ident_sb)
                nc.vector.tensor_copy(
                    cT_sb[hp * 64:(hp + 1) * 64, o, :],
                    trp[0:64, :, :].rearrange("p a b -> p (a b)"))

            def phase2_steps(o, pts):
                """Thunks: 8 ctx units + 2 transposes for pair-stage o."""
                c_sb = stg.tile([128, 2, SO, 64], bf16, name=f"c{o}", tag="c",
                                bufs=2)
                for hp in range(2):
                    for lt in range(SO):
                        yield lambda hp=hp, lt=lt: phase2_unit(
                            o, pts, hp, lt, c_sb)
                    yield lambda hp=hp: transpose_pair(o, hp, c_sb)

            # ---- 4 head-pair stages ----
            prev_p2 = None   # phase2 step iterator of the previous stage
            for o in range(SO):
                fill = mk_fill(o)
                fi = 0
                ptA = ptp.tile([128, NT, 512], bf16, name=f"ptA{o}",
                               tag="ptA", bufs=2)
                ptB = ptp.tile([128, NT, 512], bf16, name=f"ptB{o}",
                               tag="ptB", bufs=2)
                for t in range(NT):
                    stA = ps.tile([128, 512], f32, name=f"stA{o}_{t}",
                                  tag="stA", bufs=2)
                    stB = ps.tile([128, 512], f32, name=f"stB{o}_{t}",
                                  tag="stB", bufs=2)
                    tsl = slice(t * 128, (t + 1) * 128)
                    nc.tensor.matmul(stA, KT_sb[0:64, o, tsl],
                                     QT_sb[0:64, o, :], start=True, stop=True)
                    nc.tensor.matmul(stB, KT_sb[64:128, o, tsl],
                                     QT_sb[64:128, o, :], start=True, stop=True)
                    nc.scalar.activation(ptA[:, t, :], stA, EXP)
                    nc.scalar.activation(ptB[:, t, :], stB, EXP)
                    # one phase2 step of the previous stage every other t
                    if t % 2 == 1 and prev_p2 is not None:
                        step = next(prev_p2, None)
                        if step is not None:
                            step()
                        if t == NT - 1:  # 10 steps total, drain leftovers
                            for step in prev_p2:
                                step()
                    # V projection for this pair, consumed by phase2(o)
                    # which runs during stage o+1
                    vproj(o, t)
                    # round-robin deferred projections for the next stage
                    for _ in range(2):
                        if fi < len(fill):
                            fill[fi]()
                            fi += 1
                while fi < len(fill):
                    fill[fi]()
                    fi += 1
                prev_p2 = phase2_steps(o, (ptA, ptB))
            for step in prev_p2:
                step()

            # ---- out projection: out[lq, d] += cT[:, o, lq-sl].T @ wo ----
            for lt in range(SO):
                lsl = slice(lt * 128, (lt + 1) * 128)
                for dc in range(2):
                    dsl = slice(dc * 512, (dc + 1) * 512)
                    ops = ps.tile([128, 512], f32, name=f"ops{lt}_{dc}",
                                  tag="proj", bufs=2)
                    for o in range(SO):
                        nc.tensor.matmul(ops, cT_sb[:, o, lsl],
                                         wo_sb[:, o, dsl],
                                         start=(o == 0), stop=(o == SO - 1))
                    ot = ost.tile([128, 512], f32, name="ot", tag="ot")
                    nc.scalar.activation(ot, ops, IDENT)
                    nc.sync.dma_start(out[lt, :, dsl], ot)

    return _split_multi_waits(nc)


_NC = None


def _get_nc():
    global _NC
    if _NC is None:
        _NC = _build()
    return _NC


def _shard(q, kv, Wq, bq, Wk, bk, Wv, bv, Wo, bo):
    b16 = ml_dtypes.bfloat16

    def lay(a2d, co):  # [co*128, F] -> [128, co, F]
        F = a2d.shape[1]
        return np.ascontiguousarray(
            a2d.reshape(co, 128, F).transpose(1, 0, 2)).astype(b16)

    idn = np.eye(128, dtype=b16)
    in_maps = []
    for core in range(8):
        b, g = core // 2, core % 2
        sl = slice(g * S, (g + 1) * S)
        m = {
            "qT": lay(np.ascontiguousarray(q[b].T), CO),
            "kvT": lay(np.ascontiguousarray(kv[b].T), CO),
            "wqT": lay(np.ascontiguousarray((Wq[sl] * 0.125).T), CO),
            "wkT": lay(np.ascontiguousarray(Wk[sl].T), CO),
            "wvT": lay(np.ascontiguousarray(Wv[sl].T), CO),
            "woT": lay(np.ascontiguousarray(Wo[:, sl].T), SO),
            "bq": np.ascontiguousarray(
                (bq[sl] * 0.125).reshape(SO, 128).T).astype(np.float32),
            "bk": np.ascontiguousarray(
                bk[sl].reshape(SO, 128).T).astype(np.float32),
            "bvbc": np.broadcast_to(
                bv[sl].astype(b16), (128, S)).copy(),
            "ident": idn,
        }
        in_maps.append(m)
    return in_maps


def _run(in_maps, trace=False):
    res = run_bass_kernel_spmd(_get_nc(), in_maps, core_ids=list(range(8)),
                               trace=trace)
    return res


def kernel(q, kv, Wq, bq, Wk, bk, Wv, bv, Wo, bo, _trace=False):
    q, kv = np.asarray(q, np.float32), np.asarray(kv, np.float32)
    Wq, Wk = np.asarray(Wq, np.float32), np.asarray(Wk, np.float32)
    Wv, Wo = np.asarray(Wv, np.float32), np.asarray(Wo, np.float32)
    bq, bk = np.asarray(bq, np.float32), np.asarray(bk, np.float32)
    bv, bo = np.asarray(bv, np.float32), np.asarray(bo, np.float32)

    in_maps = _shard(q, kv, Wq, bq, Wk, bk, Wv, bv, Wo, bo)
    res = _run(in_maps, trace=_trace)
    B = q.shape[0]
    outp = np.empty((B, LQ, D), np.float32)
    for b in range(B):
        p0 = res.results[2 * b]["out"].reshape(LQ, D)
        p1 = res.results[2 * b + 1]["out"].reshape(LQ, D)
        outp[b] = p0 + p1 + bo[None, :]
    if _trace:
        kernel._last_exec_ns = res.exec_time_ns
        kernel._last_trace = res.instructions_and_trace
    return outp


# revision 10
# speedup vs baseline: 1.7714x; 1.0305x over previous
"""Cross-attention kernel for TRN2, 8 NeuronCores.

Sharding: core c -> (batch b = c//2, head-group g = c%2).  Each head-group is
8 heads = 512 of the 1024 d_model channels.  Within a core everything runs in
one fused software pipeline over 4 head-pair stages (o = 0..3):

  QT = wq_g.T @ q.T  (scale folded)       [512, 512]   (s, lq)
  KT = wk_g.T @ kv.T                      [512, 2048]  (s, lkv)
  V  = kv @ wv_g.T                        [2048, 512]  (lkv, s)   + ones col
  phase1(o), t = 0..15:
               ST = Kh.T-contract @ QT    [128, 512]   (lkv-tile, lq)
               P[t] = exp(ST)          -> bf16 SBUF [128, 16, 512] per head
  phase2(o), unit (hp, lt):  16 consecutive matmuls in ONE psum bank
               ctx[lq, 65] += P[t]_lt.T @ [Vh | 1]     (F=65 transposed form;
                                                        col 64 = softmax denom)
               C = ctx[:, 0:64] * recip(ctx[:, 64])  (DVE per-partition scalar)
  transpose C -> cT[s, lq]  (PE transpose, identity trick)
  out = cT.T @ wo_g.T                     [512, 1024]
Host sums the two head-group partials per batch and adds bo.

All operands are bf16 (1 cyc/row on PE at any free size; halves DMA), psum
accumulation f32.  phase2(o-1) and the projection matmuls of stage o+1 are
hand-interleaved into phase1(o)'s t-loop so the Act engine's exp stream (the
second largest engine load) fully overlaps PE work.  A psum accumulation
group owns its whole 2KB bank (start zeroes the full zero-region), hence the
consecutive-16 structure of phase2 rather than round-robin accumulation.
"""

import sys
if "/opt/trn_rl_repo" not in sys.path:
    sys.path.insert(0, "/opt/trn_rl_repo")

import numpy as np
import ml_dtypes

import concourse.bass as bass
import concourse.mybir as mybir
import concourse.tile as tile
from concourse.bass_utils import run_bass_kernel_spmd

f32 = mybir.dt.float32
bf16 = mybir.dt.bfloat16
EXP = mybir.ActivationFunctionType.Exp
IDENT = mybir.ActivationFunctionType.Identity

D = 1024        # d_model
S = 512         # per-core channel shard (8 heads x 64)
LQ = 512
LKV = 2048
CO = D // 128   # 8 contraction chunks
SO = S // 128   # 4 shard s-tiles (head pairs)
NT = LKV // 128  # 16 lkv tiles


def _split_multi_waits(nc, max_waits=1):
    """This container's walrus allows only `max_waits` sync-wait commands per
    instruction; hoist the excess into standalone EventSemaphore insts."""
    ev_id = 0
    for f in nc.m.functions:
        for bb in f.blocks:
            new = []
            changed = False
            for inst in bb.instructions:
                si = inst.sync_info
                if si is not None and si.on_wait and len(si.on_wait) > max_waits:
                    waits = list(si.on_wait)
                    for sw in waits[:-max_waits]:
                        ev = mybir.InstEventSemaphore(
                            name=f"EVSPLIT-{ev_id}", engine=inst.engine,
                            sync_info=mybir.SyncInfo(on_wait=[sw], on_update=[]))
                        ev_id += 1
                        nc.register_instruction(ev, overwrite=True)
                        new.append(ev)
                    inst.sync_info = mybir.SyncInfo(
                        on_wait=waits[-max_waits:], on_update=list(si.on_update))
                    changed = True
                new.append(inst)
            if changed:
                bb.instructions = new
    return nc


def _build():
    nc = bass.Bass(trn_type="TRN2")

    # DRAM I/O (pre-laid-out [128, outer, free] on host, bf16)
    qT = nc.dram_tensor("qT", [128, CO, LQ], bf16, kind="ExternalInput")
    kvT = nc.dram_tensor("kvT", [128, CO, LKV], bf16, kind="ExternalInput")
    wqT = nc.dram_tensor("wqT", [128, CO, S], bf16, kind="ExternalInput")
    wkT = nc.dram_tensor("wkT", [128, CO, S], bf16, kind="ExternalInput")
    wvT = nc.dram_tensor("wvT", [128, CO, S], bf16, kind="ExternalInput")
    woT = nc.dram_tensor("woT", [128, SO, D], bf16, kind="ExternalInput")
    bq = nc.dram_tensor("bq", [128, SO], f32, kind="ExternalInput")
    bk = nc.dram_tensor("bk", [128, SO], f32, kind="ExternalInput")
    bvbc = nc.dram_tensor("bvbc", [128, S], bf16, kind="ExternalInput")
    ident = nc.dram_tensor("ident", [128, 128], bf16, kind="ExternalInput")
    out = nc.dram_tensor("out", [SO, 128, D], f32, kind="ExternalOutput")

    with tile.TileContext(nc) as tc:
        with tc.tile_pool(name="wgt", bufs=1) as wgt, \
             tc.tile_pool(name="pt", bufs=2) as ptp, \
             tc.tile_pool(name="stg", bufs=4) as stg, \
             tc.tile_pool(name="ost", bufs=3) as ost, \
             tc.tile_pool(name="ps", bufs=1, space="PSUM") as ps:

            # ---- resident SBUF ----
            kv_sb = wgt.tile([128, CO, LKV], bf16, name="kv_sb")
            wk_sb = wgt.tile([128, CO, S], bf16, name="wk_sb")
            wv_sb = wgt.tile([128, CO, S], bf16, name="wv_sb")
            wq_sb = wgt.tile([128, CO, S], bf16, name="wq_sb")
            wo_sb = wgt.tile([128, SO, D], bf16, name="wo_sb")
            qT_sb = wgt.tile([128, CO, LQ], bf16, name="qT_sb")
            QT_sb = wgt.tile([128, SO, LQ], bf16, name="QT_sb")
            KT_sb = wgt.tile([128, SO, LKV], bf16, name="KT_sb")
            # V per head with a ones column: attn@V (transposed form) then
            # also yields the softmax denominator in output col 64.
            Vp_sb = wgt.tile([128, NT, 8, 65], bf16, name="Vp_sb")
            cT_sb = wgt.tile([128, SO, LQ], bf16, name="cT_sb")
            bq_sb = wgt.tile([128, SO], f32, name="bq_sb")
            bk_sb = wgt.tile([128, SO], f32, name="bk_sb")
            bvbc_sb = wgt.tile([128, S], bf16, name="bvbc_sb")
            ident_sb = wgt.tile([128, 128], bf16, name="ident_sb")

            # ---- DMA order = priority order (SP queue is serial).
            # Stage 0 only needs the o=0 slices of wq/wk, then the kv chunks
            # pace the stage-0 score loop; everything else arrives later.
            nc.sync.dma_start(bq_sb, bq[:])
            nc.sync.dma_start(bk_sb, bk[:])
            nc.sync.dma_start(bvbc_sb, bvbc[:])
            nc.sync.dma_start(ident_sb, ident[:])
            nc.sync.dma_start(wk_sb[:, :, 0:128], wkT[:, :, 0:128])
            nc.sync.dma_start(wq_sb[:, :, 0:128], wqT[:, :, 0:128])
            nc.sync.dma_start(qT_sb, qT[:])
            for ch in range(4):
                nc.sync.dma_start(kv_sb[:, :, ch * 512:(ch + 1) * 512],
                                  kvT[:, :, ch * 512:(ch + 1) * 512])
            nc.sync.dma_start(wv_sb, wvT[:])
            nc.sync.dma_start(wk_sb[:, :, 128:512], wkT[:, :, 128:512])
            nc.sync.dma_start(wq_sb[:, :, 128:512], wqT[:, :, 128:512])
            nc.sync.dma_start(wo_sb, woT[:])

            nc.vector.memset(Vp_sb[:, :, :, 64:65], 1.0)

            # ---- emission helpers (each emits PE matmuls + its drain) ----
            def kproj(o, ch):
                kps = ps.tile([128, 512], f32, name=f"kps{o}_{ch}", tag="proj",
                              bufs=2)
                sl = slice(ch * 512, (ch + 1) * 512)
                for c in range(CO):
                    nc.tensor.matmul(kps, wk_sb[:, c, o * 128:(o + 1) * 128],
                                     kv_sb[:, c, sl],
                                     start=(c == 0), stop=(c == CO - 1))
                nc.vector.tensor_scalar_add(KT_sb[:, o, sl], kps,
                                            bk_sb[:, o:o + 1])

            def qproj(o):
                qps = ps.tile([128, 512], f32, name=f"qps{o}", tag="proj",
                              bufs=2)
                for c in range(CO):
                    nc.tensor.matmul(qps, wq_sb[:, c, o * 128:(o + 1) * 128],
                                     qT_sb[:, c, :],
                                     start=(c == 0), stop=(c == CO - 1))
                nc.vector.tensor_scalar_add(QT_sb[:, o, :], qps,
                                            bq_sb[:, o:o + 1])

            def vproj(o, t):
                vps = ps.tile([128, 128], f32, name=f"vps{o}_{t}", tag="proj",
                              bufs=2)
                tsl = slice(t * 128, (t + 1) * 128)
                osl = slice(o * 128, (o + 1) * 128)
                for c in range(CO):
                    nc.tensor.matmul(vps, kv_sb[:, c, tsl], wv_sb[:, c, osl],
                                     start=(c == 0), stop=(c == CO - 1))
                nc.vector.tensor_add(
                    Vp_sb[:, t, 2 * o:2 * o + 2, 0:64],
                    vps.rearrange("p (h d) -> p h d", h=2),
                    bvbc_sb[:, osl].rearrange("p (h d) -> p h d", h=2))

            # ---- lead-in: stage-0 prerequisites ----
            qproj(0)
            kproj(0, 0)

            # Per-stage fill schedules: iteration t -> thunks.  Placement
            # matches DMA arrival order (PE is in-order, so emitting a matmul
            # whose DMA lands late would stall everything behind it).
            def mk_sched(o):
                s = {t: [] for t in range(NT)}
                if o == 0:
                    # kv chunks land one per ~3.2us; kproj(0,ch) feeds the
                    # scores at t=4ch.  wv lands after kv3.
                    s[0].append(lambda: kproj(0, 1))
                    s[4].append(lambda: kproj(0, 2))
                    s[8].append(lambda: kproj(0, 3))
                    for t in range(8, NT):
                        s[t].append(lambda t=t: vproj(0, 2 * (t - 8)))
                        s[t].append(lambda t=t: vproj(0, 2 * (t - 8) + 1))
                else:
                    # own K chunks 1..3 first (ch0/qproj ran at the tail of
                    # the previous stage), V tiles just-in-time for phase2.
                    for ch in range(1, 4):
                        s[ch - 1].append(lambda ch=ch: kproj(o, ch))
                    for t in range(NT):
                        s[t].append(lambda t=t: vproj(o, t))
                if o < 3:
                    # next stage's Q and first K chunk at the stage tail
                    s[NT - 2].append(lambda: qproj(o + 1))
                    s[NT - 1].append(lambda: kproj(o + 1, 0))
                return s

            def phase2_unit(o, pt, hp, lt, c_sb):
                """ctx unit (head hp of pair o, lq tile lt): 16 consecutive
                matmuls in one psum bank, then normalize straight from psum.
                (An accumulation group owns its whole 2KB zero-region, so the
                16 steps must be consecutive in one dedicated bank.)
                Pair 3 runs at the kernel tail where Act is idle, so its
                normalize goes to the scalar engine instead of DVE."""
                ctx = ps.tile([128, 65], f32, name=f"ctx{o}_{hp}_{lt}",
                              tag="ctx", bufs=2)
                base = hp * 512 + lt * 128
                for t in range(NT):
                    nc.tensor.matmul(
                        ctx, pt[:, t, base:base + 128],
                        Vp_sb[:, t, 2 * o + hp, :],
                        start=(t == 0), stop=(t == NT - 1))
                rc = stg.tile([128, 1], f32, name=f"rc{o}_{hp}_{lt}", tag="rc",
                              bufs=4)
                nc.vector.reciprocal(rc, ctx[:, 64:65])
                if o == SO - 1:
                    nc.scalar.activation(c_sb[:, hp, lt, :], ctx[:, 0:64],
                                         IDENT, scale=rc)
                else:
                    nc.vector.tensor_scalar_mul(
                        c_sb[:, hp, lt, :], ctx[:, 0:64], rc)

            def transpose_pair(o, hp, c_sb):
                trp = ps.tile([128, SO, 128], bf16, name=f"trp{o}_{hp}",
                              tag="proj", bufs=2)
                for lt in range(SO):
                    nc.tensor.transpose(trp[0:64, lt, :],
                                        c_sb[:, hp, lt, :], ident_sb)
                nc.vector.tensor_copy(
                    cT_sb[hp * 64:(hp + 1) * 64, o, :],
                    trp[0:64, :, :].rearrange("p a b -> p (a b)"))

            def phase2_steps(o, pt):
                """Thunks: 8 ctx units + 2 transposes for pair-stage o."""
                c_sb = stg.tile([128, 2, SO, 64], bf16, name=f"c{o}", tag="c",
                                bufs=2)
                for hp in range(2):
                    for lt in range(SO):
                        yield lambda hp=hp, lt=lt: phase2_unit(
                            o, pt, hp, lt, c_sb)
                    yield lambda hp=hp: transpose_pair(o, hp, c_sb)

            # ---- 4 head-pair stages ----
            prev_p2 = None   # phase2 step iterator of the previous stage
            for o in range(SO):
                sched = mk_sched(o)
                pt = ptp.tile([128, NT, 1024], bf16, name=f"pt{o}",
                              tag="pt", bufs=2)
                for t in range(NT):
                    # fused score tile: head 2o in bank cols 0:512, head
                    # 2o+1 in 512:1024 (each matmul stays within one bank)
                    st2 = ps.tile([128, 1024], f32, name=f"st{o}_{t}",
                                  tag="st", bufs=2)
                    tsl = slice(t * 128, (t + 1) * 128)
                    nc.tensor.matmul(st2[:, 0:512], KT_sb[0:64, o, tsl],
                                     QT_sb[0:64, o, :], start=True, stop=True)
                    nc.tensor.matmul(st2[:, 512:1024], KT_sb[64:128, o, tsl],
                                     QT_sb[64:128, o, :], start=True, stop=True)
                    nc.scalar.activation(pt[:, t, :], st2, EXP)
                    # one phase2 step of the previous stage every other t
                    if t % 2 == 1 and prev_p2 is not None:
                        step = next(prev_p2, None)
                        if step is not None:
                            step()
                        if t == NT - 1:  # 10 steps total, drain leftovers
                            for step in prev_p2:
                                step()
                    for thunk in sched[t]:
                        thunk()
                prev_p2 = phase2_steps(o, pt)
            for step in prev_p2:
                step()

            # ---- out projection: out[lq, d] += cT[:, o, lq-sl].T @ wo ----
            for lt in range(SO):
                lsl = slice(lt * 128, (lt + 1) * 128)
                for dc in range(2):
                    dsl = slice(dc * 512, (dc + 1) * 512)
                    ops = ps.tile([128, 512], f32, name=f"ops{lt}_{dc}",
                                  tag="proj", bufs=2)
                    for o in range(SO):
                        nc.tensor.matmul(ops, cT_sb[:, o, lsl],
                                         wo_sb[:, o, dsl],
                                         start=(o == 0), stop=(o == SO - 1))
                    ot = ost.tile([128, 512], f32, name="ot", tag="ot")
                    nc.scalar.activation(ot, ops, IDENT)
                    nc.sync.dma_start(out[lt, :, dsl], ot)

    return _split_multi_waits(nc)


_NC = None


def _get_nc():
    global _NC
    if _NC is None:
        _NC = _build()
    return _NC


def _shard(q, kv, Wq, bq, Wk, bk, Wv, bv, Wo, bo):
    b16 = ml_dtypes.bfloat16

    def lay(a2d, co):  # [co*128, F] -> [128, co, F]
        F = a2d.shape[1]
        return np.ascontiguousarray(
            a2d.reshape(co, 128, F).transpose(1, 0, 2)).astype(b16)

    idn = np.eye(128, dtype=b16)
    in_maps = []
    for core in range(8):
        b, g = core // 2, core % 2
        sl = slice(g * S, (g + 1) * S)
        m = {
            "qT": lay(np.ascontiguousarray(q[b].T), CO),
            "kvT": lay(np.ascontiguousarray(kv[b].T), CO),
            "wqT": lay(np.ascontiguousarray((Wq[sl] * 0.125).T), CO),
            "wkT": lay(np.ascontiguousarray(Wk[sl].T), CO),
            "wvT": lay(np.ascontiguousarray(Wv[sl].T), CO),
            "woT": lay(np.ascontiguousarray(Wo[:, sl].T), SO),
            "bq": np.ascontiguousarray(
                (bq[sl] * 0.125).reshape(SO, 128).T).astype(np.float32),
            "bk": np.ascontiguousarray(
                bk[sl].reshape(SO, 128).T).astype(np.float32),
            "bvbc": np.broadcast_to(
                bv[sl].astype(b16), (128, S)).copy(),
            "ident": idn,
        }
        in_maps.append(m)
    return in_maps


def _run(in_maps, trace=False):
    res = run_bass_kernel_spmd(_get_nc(), in_maps, core_ids=list(range(8)),
                               trace=trace)
    return res


def kernel(q, kv, Wq, bq, Wk, bk, Wv, bv, Wo, bo, _trace=False):
    q, kv = np.asarray(q, np.float32), np.asarray(kv, np.float32)
    Wq, Wk = np.asarray(Wq, np.float32), np.asarray(Wk, np.float32)
    Wv, Wo = np.asarray(Wv, np.float32), np.asarray(Wo, np.float32)
    bq, bk = np.asarray(bq, np.float32), np.asarray(bk, np.float32)
    bv, bo = np.asarray(bv, np.float32), np.asarray(bo, np.float32)

    in_maps = _shard(q, kv, Wq, bq, Wk, bk, Wv, bv, Wo, bo)
    res = _run(in_maps, trace=_trace)
    B = q.shape[0]
    outp = np.empty((B, LQ, D), np.float32)
    for b in range(B):
        p0 = res.results[2 * b]["out"].reshape(LQ, D)
        p1 = res.results[2 * b + 1]["out"].reshape(LQ, D)
        outp[b] = p0 + p1 + bo[None, :]
    if _trace:
        kernel._last_exec_ns = res.exec_time_ns
        kernel._last_trace = res.instructions_and_trace
    return outp


# revision 12
# speedup vs baseline: 1.8003x; 1.0163x over previous
"""Cross-attention kernel for TRN2, 8 NeuronCores.

Sharding: core c -> (batch b = c//2, head-group g = c%2).  Each head-group is
8 heads = 512 of the 1024 d_model channels.  Within a core everything runs in
one fused software pipeline over 4 head-pair stages (o = 0..3):

  QT = wq_g.T @ q.T  (scale folded)       [512, 512]   (s, lq)
  KT = wk_g.T @ kv.T                      [512, 2048]  (s, lkv)
  V  = kv @ wv_g.T                        [2048, 512]  (lkv, s)   + ones col
  phase1(o), t = 0..15:
               ST = Kh.T-contract @ QT    [128, 512]   (lkv-tile, lq)
               P[t] = exp(ST)          -> bf16 SBUF [128, 16, 512] per head
  phase2(o), unit (hp, lt):  16 consecutive matmuls in ONE psum bank
               ctx[lq, 65] += P[t]_lt.T @ [Vh | 1]     (F=65 transposed form;
                                                        col 64 = softmax denom)
               C = ctx[:, 0:64] * recip(ctx[:, 64])  (DVE per-partition scalar)
  transpose C -> cT[s, lq]  (PE transpose, identity trick)
  out = cT.T @ wo_g.T                     [512, 1024]
Host sums the two head-group partials per batch and adds bo.

All operands are bf16 (1 cyc/row on PE at any free size; halves DMA), psum
accumulation f32.  phase2(o-1) and the projection matmuls of stage o+1 are
hand-interleaved into phase1(o)'s t-loop so the Act engine's exp stream (the
second largest engine load) fully overlaps PE work.  A psum accumulation
group owns its whole 2KB bank (start zeroes the full zero-region), hence the
consecutive-16 structure of phase2 rather than round-robin accumulation.
"""

import sys
if "/opt/trn_rl_repo" not in sys.path:
    sys.path.insert(0, "/opt/trn_rl_repo")

import numpy as np
import ml_dtypes

import concourse.bass as bass
import concourse.mybir as mybir
import concourse.tile as tile
from concourse.bass_utils import run_bass_kernel_spmd

f32 = mybir.dt.float32
bf16 = mybir.dt.bfloat16
EXP = mybir.ActivationFunctionType.Exp
IDENT = mybir.ActivationFunctionType.Identity

D = 1024        # d_model
S = 512         # per-core channel shard (8 heads x 64)
LQ = 512
LKV = 2048
CO = D // 128   # 8 contraction chunks
SO = S // 128   # 4 shard s-tiles (head pairs)
NT = LKV // 128  # 16 lkv tiles


def _split_multi_waits(nc, max_waits=1):
    """This container's walrus allows only `max_waits` sync-wait commands per
    instruction; hoist the excess into standalone EventSemaphore insts."""
    ev_id = 0
    for f in nc.m.functions:
        for bb in f.blocks:
            new = []
            changed = False
            for inst in bb.instructions:
                si = inst.sync_info
                if si is not None and si.on_wait and len(si.on_wait) > max_waits:
                    waits = list(si.on_wait)
                    for sw in waits[:-max_waits]:
                        ev = mybir.InstEventSemaphore(
                            name=f"EVSPLIT-{ev_id}", engine=inst.engine,
                            sync_info=mybir.SyncInfo(on_wait=[sw], on_update=[]))
                        ev_id += 1
                        nc.register_instruction(ev, overwrite=True)
                        new.append(ev)
                    inst.sync_info = mybir.SyncInfo(
                        on_wait=waits[-max_waits:], on_update=list(si.on_update))
                    changed = True
                new.append(inst)
            if changed:
                bb.instructions = new
    return nc


def _build():
    nc = bass.Bass(trn_type="TRN2")

    # DRAM I/O (pre-laid-out [128, outer, free] on host, bf16)
    qT = nc.dram_tensor("qT", [128, CO, LQ], bf16, kind="ExternalInput")
    kvT = nc.dram_tensor("kvT", [128, CO, LKV], bf16, kind="ExternalInput")
    wqT = nc.dram_tensor("wqT", [128, CO, S], bf16, kind="ExternalInput")
    wkT = nc.dram_tensor("wkT", [128, CO, S], bf16, kind="ExternalInput")
    wvT = nc.dram_tensor("wvT", [128, CO, S], bf16, kind="ExternalInput")
    woT = nc.dram_tensor("woT", [128, SO, D], bf16, kind="ExternalInput")
    bq = nc.dram_tensor("bq", [128, SO], f32, kind="ExternalInput")
    bk = nc.dram_tensor("bk", [128, SO], f32, kind="ExternalInput")
    bvbc = nc.dram_tensor("bvbc", [128, S], bf16, kind="ExternalInput")
    ident = nc.dram_tensor("ident", [128, 128], bf16, kind="ExternalInput")
    out = nc.dram_tensor("out", [SO, 128, D], f32, kind="ExternalOutput")

    with tile.TileContext(nc) as tc:
        with tc.tile_pool(name="wgt", bufs=1) as wgt, \
             tc.tile_pool(name="pt", bufs=2) as ptp, \
             tc.tile_pool(name="stg", bufs=4) as stg, \
             tc.tile_pool(name="ost", bufs=3) as ost, \
             tc.tile_pool(name="ps", bufs=1, space="PSUM") as ps:

            # ---- resident SBUF ----
            kv_sb = wgt.tile([128, CO, LKV], bf16, name="kv_sb")
            wk_sb = wgt.tile([128, CO, S], bf16, name="wk_sb")
            wv_sb = wgt.tile([128, CO, S], bf16, name="wv_sb")
            wq_sb = wgt.tile([128, CO, S], bf16, name="wq_sb")
            wo_sb = wgt.tile([128, SO, D], bf16, name="wo_sb")
            qT_sb = wgt.tile([128, CO, LQ], bf16, name="qT_sb")
            QT_sb = wgt.tile([128, SO, LQ], bf16, name="QT_sb")
            KT_sb = wgt.tile([128, SO, LKV], bf16, name="KT_sb")
            # V per head with a ones column: attn@V (transposed form) then
            # also yields the softmax denominator in output col 64.
            Vp_sb = wgt.tile([128, NT, 8, 65], bf16, name="Vp_sb")
            cT_sb = wgt.tile([128, SO, LQ], bf16, name="cT_sb")
            bq_sb = wgt.tile([128, SO], f32, name="bq_sb")
            bk_sb = wgt.tile([128, SO], f32, name="bk_sb")
            bvbc_sb = wgt.tile([128, S], bf16, name="bvbc_sb")
            ident_sb = wgt.tile([128, 128], bf16, name="ident_sb")

            # ---- DMA order = priority order (SP queue is serial).
            # Stage 0 only needs the o=0 slices of wq/wk, then the kv chunks
            # pace the stage-0 score loop; everything else arrives later.
            nc.sync.dma_start(bq_sb, bq[:])
            nc.sync.dma_start(bk_sb, bk[:])
            nc.sync.dma_start(bvbc_sb, bvbc[:])
            nc.sync.dma_start(ident_sb, ident[:])
            nc.sync.dma_start(wk_sb[:, :, 0:128], wkT[:, :, 0:128])
            nc.sync.dma_start(wq_sb[:, :, 0:128], wqT[:, :, 0:128])
            nc.sync.dma_start(qT_sb, qT[:])
            for hh in range(8):
                nc.sync.dma_start(kv_sb[:, 4 * (hh % 2):4 * (hh % 2) + 4,
                                        (hh // 2) * 512:(hh // 2 + 1) * 512],
                                  kvT[:, 4 * (hh % 2):4 * (hh % 2) + 4,
                                      (hh // 2) * 512:(hh // 2 + 1) * 512])
            nc.sync.dma_start(wv_sb, wvT[:])
            nc.sync.dma_start(wk_sb[:, :, 128:512], wkT[:, :, 128:512])
            nc.sync.dma_start(wq_sb[:, :, 128:512], wqT[:, :, 128:512])
            nc.sync.dma_start(wo_sb, woT[:])

            nc.vector.memset(Vp_sb[:, :, :, 64:65], 1.0)

            # ---- emission helpers (each emits PE matmuls + its drain) ----
            def kproj(o, ch):
                kps = ps.tile([128, 512], f32, name=f"kps{o}_{ch}", tag="proj",
                              bufs=2)
                sl = slice(ch * 512, (ch + 1) * 512)
                for c in range(CO):
                    nc.tensor.matmul(kps, wk_sb[:, c, o * 128:(o + 1) * 128],
                                     kv_sb[:, c, sl],
                                     start=(c == 0), stop=(c == CO - 1))
                nc.vector.tensor_scalar_add(KT_sb[:, o, sl], kps,
                                            bk_sb[:, o:o + 1])

            def qproj(o):
                qps = ps.tile([128, 512], f32, name=f"qps{o}", tag="proj",
                              bufs=2)
                for c in range(CO):
                    nc.tensor.matmul(qps, wq_sb[:, c, o * 128:(o + 1) * 128],
                                     qT_sb[:, c, :],
                                     start=(c == 0), stop=(c == CO - 1))
                nc.vector.tensor_scalar_add(QT_sb[:, o, :], qps,
                                            bq_sb[:, o:o + 1])

            def vproj(o, t):
                vps = ps.tile([128, 128], f32, name=f"vps{o}_{t}", tag="proj",
                              bufs=2)
                tsl = slice(t * 128, (t + 1) * 128)
                osl = slice(o * 128, (o + 1) * 128)
                for c in range(CO):
                    nc.tensor.matmul(vps, kv_sb[:, c, tsl], wv_sb[:, c, osl],
                                     start=(c == 0), stop=(c == CO - 1))
                nc.vector.tensor_add(
                    Vp_sb[:, t, 2 * o:2 * o + 2, 0:64],
                    vps.rearrange("p (h d) -> p h d", h=2),
                    bvbc_sb[:, osl].rearrange("p (h d) -> p h d", h=2))

            # ---- lead-in: stage-0 prerequisites ----
            qproj(0)
            kproj(0, 0)

            # Per-stage fill schedules: iteration t -> thunks.  Placement
            # matches DMA arrival order (PE is in-order, so emitting a matmul
            # whose DMA lands late would stall everything behind it).
            def mk_sched(o):
                s = {t: [] for t in range(NT)}
                if o == 0:
                    # kv chunks land one per ~3.2us; kproj(0,ch) feeds the
                    # scores at t=4ch.  wv lands after kv3.
                    s[0].append(lambda: kproj(0, 1))
                    s[4].append(lambda: kproj(0, 2))
                    s[8].append(lambda: kproj(0, 3))
                    for t in range(8, NT):
                        s[t].append(lambda t=t: vproj(0, 2 * (t - 8)))
                        s[t].append(lambda t=t: vproj(0, 2 * (t - 8) + 1))
                else:
                    # own K chunks 1..3 first (ch0/qproj ran at the tail of
                    # the previous stage), V tiles just-in-time for phase2.
                    for ch in range(1, 4):
                        s[ch - 1].append(lambda ch=ch: kproj(o, ch))
                    for t in range(NT):
                        s[t].append(lambda t=t: vproj(o, t))
                if o < 3:
                    # next stage's Q and first K chunk at the stage tail
                    s[NT - 2].append(lambda: qproj(o + 1))
                    s[NT - 1].append(lambda: kproj(o + 1, 0))
                return s

            def phase2_unit(o, pt, hp, lt, c_sb):
                """ctx unit (head hp of pair o, lq tile lt): 16 consecutive
                matmuls in one psum bank, then normalize straight from psum.
                (An accumulation group owns its whole 2KB zero-region, so the
                16 steps must be consecutive in one dedicated bank.)
                Pair 3 runs at the kernel tail where Act is idle, so its
                normalize goes to the scalar engine instead of DVE."""
                ctx = ps.tile([128, 65], f32, name=f"ctx{o}_{hp}_{lt}",
                              tag="ctx", bufs=2)
                base = hp * 512 + lt * 128
                for t in range(NT):
                    nc.tensor.matmul(
                        ctx, pt[:, t, base:base + 128],
                        Vp_sb[:, t, 2 * o + hp, :],
                        start=(t == 0), stop=(t == NT - 1))
                rc = stg.tile([128, 1], f32, name=f"rc{o}_{hp}_{lt}", tag="rc",
                              bufs=4)
                nc.vector.reciprocal(rc, ctx[:, 64:65])
                if o == SO - 1:
                    nc.scalar.activation(c_sb[:, hp, lt, :], ctx[:, 0:64],
                                         IDENT, scale=rc)
                else:
                    nc.vector.tensor_scalar_mul(
                        c_sb[:, hp, lt, :], ctx[:, 0:64], rc)

            def transpose_pair(o, hp, c_sb):
                trp = ps.tile([128, SO, 128], bf16, name=f"trp{o}_{hp}",
                              tag="proj", bufs=2)
                for lt in range(SO):
                    nc.tensor.transpose(trp[0:64, lt, :],
                                        c_sb[:, hp, lt, :], ident_sb)
                nc.vector.tensor_copy(
                    cT_sb[hp * 64:(hp + 1) * 64, o, :],
                    trp[0:64, :, :].rearrange("p a b -> p (a b)"))

            def phase2_steps(o, pt):
                """Thunks: 8 ctx units + 2 transposes for pair-stage o."""
                c_sb = stg.tile([128, 2, SO, 64], bf16, name=f"c{o}", tag="c",
                                bufs=2)
                for hp in range(2):
                    for lt in range(SO):
                        yield lambda hp=hp, lt=lt: phase2_unit(
                            o, pt, hp, lt, c_sb)
                    yield lambda hp=hp: transpose_pair(o, hp, c_sb)

            # ---- 4 head-pair stages ----
            prev_p2 = None   # phase2 step iterator of the previous stage
            for o in range(SO):
                sched = mk_sched(o)
                pt = ptp.tile([128, NT, 1024], bf16, name=f"pt{o}",
                              tag="pt", bufs=2)
                for t in range(NT):
                    # fused score tile: head 2o in bank cols 0:512, head
                    # 2o+1 in 512:1024 (each matmul stays within one bank)
                    st2 = ps.tile([128, 1024], f32, name=f"st{o}_{t}",
                                  tag="st", bufs=2)
                    tsl = slice(t * 128, (t + 1) * 128)
                    nc.tensor.matmul(st2[:, 0:512], KT_sb[0:64, o, tsl],
                                     QT_sb[0:64, o, :], start=True, stop=True)
                    nc.tensor.matmul(st2[:, 512:1024], KT_sb[64:128, o, tsl],
                                     QT_sb[64:128, o, :], start=True, stop=True)
                    nc.scalar.activation(pt[:, t, :], st2, EXP)
                    # one phase2 step of the previous stage every other t
                    if t % 2 == 1 and prev_p2 is not None:
                        step = next(prev_p2, None)
                        if step is not None:
                            step()
                        if t == NT - 1:  # 10 steps total, drain leftovers
                            for step in prev_p2:
                                step()
                    for thunk in sched[t]:
                        thunk()
                prev_p2 = phase2_steps(o, pt)
            for step in prev_p2:
                step()

            # ---- out projection: out[lq, d] += cT[:, o, lq-sl].T @ wo ----
            # One staging tile + one DMA per lq tile (per-DMA fixed costs
            # dominate the tail otherwise).
            for lt in range(SO):
                lsl = slice(lt * 128, (lt + 1) * 128)
                ot = ost.tile([128, D], f32, name="ot", tag="ot")
                for dc in range(2):
                    dsl = slice(dc * 512, (dc + 1) * 512)
                    ops = ps.tile([128, 512], f32, name=f"ops{lt}_{dc}",
                                  tag="proj", bufs=2)
                    for o in range(SO):
                        nc.tensor.matmul(ops, cT_sb[:, o, lsl],
                                         wo_sb[:, o, dsl],
                                         start=(o == 0), stop=(o == SO - 1))
                    nc.scalar.activation(ot[:, dsl], ops, IDENT)
                nc.sync.dma_start(out[lt, :, :], ot)

    return _split_multi_waits(nc)


_NC = None


def _get_nc():
    global _NC
    if _NC is None:
        _NC = _build()
    return _NC


def _shard(q, kv, Wq, bq, Wk, bk, Wv, bv, Wo, bo):
    b16 = ml_dtypes.bfloat16

    def lay(a2d, co):  # [co*128, F] -> [128, co, F]
        F = a2d.shape[1]
        return np.ascontiguousarray(
            a2d.reshape(co, 128, F).transpose(1, 0, 2)).astype(b16)

    idn = np.eye(128, dtype=b16)
    in_maps = []
    for core in range(8):
        b, g = core // 2, core % 2
        sl = slice(g * S, (g + 1) * S)
        m = {
            "qT": lay(np.ascontiguousarray(q[b].T), CO),
            "kvT": lay(np.ascontiguousarray(kv[b].T), CO),
            "wqT": lay(np.ascontiguousarray((Wq[sl] * 0.125).T), CO),
            "wkT": lay(np.ascontiguousarray(Wk[sl].T), CO),
            "wvT": lay(np.ascontiguousarray(Wv[sl].T), CO),
            "woT": lay(np.ascontiguousarray(Wo[:, sl].T), SO),
            "bq": np.ascontiguousarray(
                (bq[sl] * 0.125).reshape(SO, 128).T).astype(np.float32),
            "bk": np.ascontiguousarray(
                bk[sl].reshape(SO, 128).T).astype(np.float32),
            "bvbc": np.broadcast_to(
                bv[sl].astype(b16), (128, S)).copy(),
            "ident": idn,
        }
        in_maps.append(m)
    return in_maps


def _run(in_maps, trace=False):
    res = run_bass_kernel_spmd(_get_nc(), in_maps, core_ids=list(range(8)),
                               trace=trace)
    return res


def kernel(q, kv, Wq, bq, Wk, bk, Wv, bv, Wo, bo, _trace=False):
    q, kv = np.asarray(q, np.float32), np.asarray(kv, np.float32)
    Wq, Wk = np.asarray(Wq, np.float32), np.asarray(Wk, np.float32)
    Wv, Wo = np.asarray(Wv, np.float32), np.asarray(Wo, np.float32)
    bq, bk = np.asarray(bq, np.float32), np.asarray(bk, np.float32)
    bv, bo = np.asarray(bv, np.float32), np.asarray(bo, np.float32)

    in_maps = _shard(q, kv, Wq, bq, Wk, bk, Wv, bv, Wo, bo)
    res = _run(in_maps, trace=_trace)
    B = q.shape[0]
    outp = np.empty((B, LQ, D), np.float32)
    for b in range(B):
        p0 = res.results[2 * b]["out"].reshape(LQ, D)
        p1 = res.results[2 * b + 1]["out"].reshape(LQ, D)
        outp[b] = p0 + p1 + bo[None, :]
    if _trace:
        kernel._last_exec_ns = res.exec_time_ns
        kernel._last_trace = res.instructions_and_trace
    return outp


# revision 13
# speedup vs baseline: 1.8500x; 1.0276x over previous
"""Cross-attention kernel for TRN2, 8 NeuronCores.

Sharding: core c -> (batch b = c//2, head-group g = c%2).  Each head-group is
8 heads = 512 of the 1024 d_model channels.  Within a core everything runs in
one fused software pipeline over 4 head-pair stages (o = 0..3):

  QT = wq_g.T @ q.T  (scale folded)       [512, 512]   (s, lq)
  KT = wk_g.T @ kv.T                      [512, 2048]  (s, lkv)
  V  = kv @ wv_g.T                        [2048, 512]  (lkv, s)   + ones col
  phase1(o), t = 0..15:
               ST = Kh.T-contract @ QT    [128, 512]   (lkv-tile, lq)
               P[t] = exp(ST)          -> bf16 SBUF [128, 16, 512] per head
  phase2(o), unit (hp, lt):  16 consecutive matmuls in ONE psum bank
               ctx[lq, 65] += P[t]_lt.T @ [Vh | 1]     (F=65 transposed form;
                                                        col 64 = softmax denom)
               C = ctx[:, 0:64] * recip(ctx[:, 64])  (DVE per-partition scalar)
  transpose C -> cT[s, lq]  (PE transpose, identity trick)
  out = cT.T @ wo_g.T                     [512, 1024]
Host sums the two head-group partials per batch and adds bo.

All operands are bf16 (1 cyc/row on PE at any free size; halves DMA), psum
accumulation f32.  phase2(o-1) and the projection matmuls of stage o+1 are
hand-interleaved into phase1(o)'s t-loop so the Act engine's exp stream (the
second largest engine load) fully overlaps PE work.  A psum accumulation
group owns its whole 2KB bank (start zeroes the full zero-region), hence the
consecutive-16 structure of phase2 rather than round-robin accumulation.
"""

import sys
if "/opt/trn_rl_repo" not in sys.path:
    sys.path.insert(0, "/opt/trn_rl_repo")

import numpy as np
import ml_dtypes

import concourse.bass as bass
import concourse.mybir as mybir
import concourse.tile as tile
from concourse.bass_utils import run_bass_kernel_spmd

f32 = mybir.dt.float32
bf16 = mybir.dt.bfloat16
EXP = mybir.ActivationFunctionType.Exp
IDENT = mybir.ActivationFunctionType.Identity

D = 1024        # d_model
S = 512         # per-core channel shard (8 heads x 64)
LQ = 512
LKV = 2048
CO = D // 128   # 8 contraction chunks
SO = S // 128   # 4 shard s-tiles (head pairs)
NT = LKV // 128  # 16 lkv tiles


def _split_multi_waits(nc, max_waits=1):
    """This container's walrus allows only `max_waits` sync-wait commands per
    instruction; hoist the excess into standalone EventSemaphore insts."""
    ev_id = 0
    for f in nc.m.functions:
        for bb in f.blocks:
            new = []
            changed = False
            for inst in bb.instructions:
                si = inst.sync_info
                if si is not None and si.on_wait and len(si.on_wait) > max_waits:
                    waits = list(si.on_wait)
                    for sw in waits[:-max_waits]:
                        ev = mybir.InstEventSemaphore(
                            name=f"EVSPLIT-{ev_id}", engine=inst.engine,
                            sync_info=mybir.SyncInfo(on_wait=[sw], on_update=[]))
                        ev_id += 1
                        nc.register_instruction(ev, overwrite=True)
                        new.append(ev)
                    inst.sync_info = mybir.SyncInfo(
                        on_wait=waits[-max_waits:], on_update=list(si.on_update))
                    changed = True
                new.append(inst)
            if changed:
                bb.instructions = new
    return nc


def _build():
    nc = bass.Bass(trn_type="TRN2")

    # DRAM I/O (pre-laid-out [128, outer, free] on host, bf16)
    qT = nc.dram_tensor("qT", [128, CO, LQ], bf16, kind="ExternalInput")
    kvT = nc.dram_tensor("kvT", [128, CO, LKV], bf16, kind="ExternalInput")
    wqT = nc.dram_tensor("wqT", [128, CO, S], bf16, kind="ExternalInput")
    wkT = nc.dram_tensor("wkT", [128, CO, S], bf16, kind="ExternalInput")
    wvT = nc.dram_tensor("wvT", [128, CO, S], bf16, kind="ExternalInput")
    woT = nc.dram_tensor("woT", [128, SO, D], bf16, kind="ExternalInput")
    bq = nc.dram_tensor("bq", [128, SO], f32, kind="ExternalInput")
    bk = nc.dram_tensor("bk", [128, SO], f32, kind="ExternalInput")
    bvbc = nc.dram_tensor("bvbc", [128, S], bf16, kind="ExternalInput")
    ident = nc.dram_tensor("ident", [128, 128], bf16, kind="ExternalInput")
    out = nc.dram_tensor("out", [SO, 128, D], f32, kind="ExternalOutput")

    with tile.TileContext(nc) as tc:
        with tc.tile_pool(name="wgt", bufs=1) as wgt, \
             tc.tile_pool(name="pt", bufs=2) as ptp, \
             tc.tile_pool(name="stg", bufs=4) as stg, \
             tc.tile_pool(name="ost", bufs=3) as ost, \
             tc.tile_pool(name="ps", bufs=1, space="PSUM") as ps:

            # ---- resident SBUF ----
            kv_sb = wgt.tile([128, CO, LKV], bf16, name="kv_sb")
            wk_sb = wgt.tile([128, CO, S], bf16, name="wk_sb")
            wv_sb = wgt.tile([128, CO, S], bf16, name="wv_sb")
            wq_sb = wgt.tile([128, CO, S], bf16, name="wq_sb")
            wo_sb = wgt.tile([128, SO, D], bf16, name="wo_sb")
            qT_sb = wgt.tile([128, CO, LQ], bf16, name="qT_sb")
            QT_sb = wgt.tile([128, SO, LQ], bf16, name="QT_sb")
            KT_sb = wgt.tile([128, SO, LKV], bf16, name="KT_sb")
            # V per head with a ones column: attn@V (transposed form) then
            # also yields the softmax denominator in output col 64.
            Vp_sb = wgt.tile([128, NT, 8, 65], bf16, name="Vp_sb")
            cT_sb = wgt.tile([128, SO, LQ], bf16, name="cT_sb")
            bq_sb = wgt.tile([128, SO], f32, name="bq_sb")
            bk_sb = wgt.tile([128, SO], f32, name="bk_sb")
            bvbc_sb = wgt.tile([128, S], bf16, name="bvbc_sb")
            ident_sb = wgt.tile([128, 128], bf16, name="ident_sb")

            # ---- DMA order = priority order (SP queue is serial).
            # Stage 0 only needs the o=0 slices of wq/wk, then the kv chunks
            # pace the stage-0 score loop; everything else arrives later.
            # Small tensors go between the chunks they are first needed after
            # (each DMA costs ~650ns of DGE issue latency regardless of size).
            nc.sync.dma_start(wk_sb[:, :, 0:128], wkT[:, :, 0:128])
            nc.sync.dma_start(wq_sb[:, :, 0:128], wqT[:, :, 0:128])
            nc.sync.dma_start(qT_sb, qT[:])
            nc.sync.dma_start(bq_sb, bq[:])
            nc.sync.dma_start(kv_sb[:, 0:4, 0:512], kvT[:, 0:4, 0:512])
            nc.sync.dma_start(kv_sb[:, 4:8, 0:512], kvT[:, 4:8, 0:512])
            nc.sync.dma_start(bk_sb, bk[:])
            for hh in range(2, 8):
                nc.sync.dma_start(kv_sb[:, 4 * (hh % 2):4 * (hh % 2) + 4,
                                        (hh // 2) * 512:(hh // 2 + 1) * 512],
                                  kvT[:, 4 * (hh % 2):4 * (hh % 2) + 4,
                                      (hh // 2) * 512:(hh // 2 + 1) * 512])
            nc.sync.dma_start(wv_sb, wvT[:])
            nc.sync.dma_start(bvbc_sb, bvbc[:])
            nc.sync.dma_start(ident_sb, ident[:])
            nc.sync.dma_start(wk_sb[:, :, 128:512], wkT[:, :, 128:512])
            nc.sync.dma_start(wq_sb[:, :, 128:512], wqT[:, :, 128:512])
            nc.sync.dma_start(wo_sb, woT[:])

            nc.vector.memset(Vp_sb[:, :, :, 64:65], 1.0)

            # ---- PE warm-up: the tensor engine runs at half clock until it
            # has been continuously busy for 3us.  Burn the initial DMA wait
            # on dummy matmuls so the real projections start at full speed.
            dm_sb = wgt.tile([128, 512], bf16, name="dm_sb")
            nc.vector.memset(dm_sb, 0.0)
            for i in range(24):
                dps = ps.tile([128, 512], f32, name=f"dps{i}", tag="proj",
                              bufs=2)
                nc.tensor.matmul(dps, dm_sb[:, 0:128], dm_sb,
                                 start=True, stop=True)

            # ---- emission helpers (each emits PE matmuls + its drain) ----
            def kproj(o, ch):
                kps = ps.tile([128, 512], f32, name=f"kps{o}_{ch}", tag="proj",
                              bufs=2)
                sl = slice(ch * 512, (ch + 1) * 512)
                for c in range(CO):
                    nc.tensor.matmul(kps, wk_sb[:, c, o * 128:(o + 1) * 128],
                                     kv_sb[:, c, sl],
                                     start=(c == 0), stop=(c == CO - 1))
                nc.vector.tensor_scalar_add(KT_sb[:, o, sl], kps,
                                            bk_sb[:, o:o + 1])

            def qproj(o):
                qps = ps.tile([128, 512], f32, name=f"qps{o}", tag="proj",
                              bufs=2)
                for c in range(CO):
                    nc.tensor.matmul(qps, wq_sb[:, c, o * 128:(o + 1) * 128],
                                     qT_sb[:, c, :],
                                     start=(c == 0), stop=(c == CO - 1))
                nc.vector.tensor_scalar_add(QT_sb[:, o, :], qps,
                                            bq_sb[:, o:o + 1])

            def vproj(o, t):
                vps = ps.tile([128, 128], f32, name=f"vps{o}_{t}", tag="proj",
                              bufs=2)
                tsl = slice(t * 128, (t + 1) * 128)
                osl = slice(o * 128, (o + 1) * 128)
                for c in range(CO):
                    nc.tensor.matmul(vps, kv_sb[:, c, tsl], wv_sb[:, c, osl],
                                     start=(c == 0), stop=(c == CO - 1))
                nc.vector.tensor_add(
                    Vp_sb[:, t, 2 * o:2 * o + 2, 0:64],
                    vps.rearrange("p (h d) -> p h d", h=2),
                    bvbc_sb[:, osl].rearrange("p (h d) -> p h d", h=2))

            # ---- lead-in: stage-0 prerequisites ----
            qproj(0)
            kproj(0, 0)

            # Per-stage fill schedules: iteration t -> thunks.  Placement
            # matches DMA arrival order (PE is in-order, so emitting a matmul
            # whose DMA lands late would stall everything behind it).
            def mk_sched(o):
                s = {t: [] for t in range(NT)}
                if o == 0:
                    # kv chunks land one per ~3.2us; kproj(0,ch) feeds the
                    # scores at t=4ch.  wv lands after kv3.
                    s[0].append(lambda: kproj(0, 1))
                    s[4].append(lambda: kproj(0, 2))
                    s[8].append(lambda: kproj(0, 3))
                    for t in range(8, NT):
                        s[t].append(lambda t=t: vproj(0, 2 * (t - 8)))
                        s[t].append(lambda t=t: vproj(0, 2 * (t - 8) + 1))
                else:
                    # own K chunks 1..3 first (ch0/qproj ran at the tail of
                    # the previous stage), V tiles just-in-time for phase2.
                    for ch in range(1, 4):
                        s[ch - 1].append(lambda ch=ch: kproj(o, ch))
                    for t in range(NT):
                        s[t].append(lambda t=t: vproj(o, t))
                if o < 3:
                    # next stage's Q and first K chunk at the stage tail
                    s[NT - 2].append(lambda: qproj(o + 1))
                    s[NT - 1].append(lambda: kproj(o + 1, 0))
                return s

            def phase2_unit(o, pt, hp, lt, c_sb):
                """ctx unit (head hp of pair o, lq tile lt): 16 consecutive
                matmuls in one psum bank, then normalize straight from psum.
                (An accumulation group owns its whole 2KB zero-region, so the
                16 steps must be consecutive in one dedicated bank.)
                Pair 3 runs at the kernel tail where Act is idle, so its
                normalize goes to the scalar engine instead of DVE."""
                ctx = ps.tile([128, 65], f32, name=f"ctx{o}_{hp}_{lt}",
                              tag="ctx", bufs=2)
                base = hp * 512 + lt * 128
                for t in range(NT):
                    nc.tensor.matmul(
                        ctx, pt[:, t, base:base + 128],
                        Vp_sb[:, t, 2 * o + hp, :],
                        start=(t == 0), stop=(t == NT - 1))
                rc = stg.tile([128, 1], f32, name=f"rc{o}_{hp}_{lt}", tag="rc",
                              bufs=4)
                nc.vector.reciprocal(rc, ctx[:, 64:65])
                if o == SO - 1:
                    nc.scalar.activation(c_sb[:, hp, lt, :], ctx[:, 0:64],
                                         IDENT, scale=rc)
                else:
                    nc.vector.tensor_scalar_mul(
                        c_sb[:, hp, lt, :], ctx[:, 0:64], rc)

            def transpose_pair(o, hp, c_sb):
                trp = ps.tile([128, SO, 128], bf16, name=f"trp{o}_{hp}",
                              tag="proj", bufs=2)
                for lt in range(SO):
                    nc.tensor.transpose(trp[0:64, lt, :],
                                        c_sb[:, hp, lt, :], ident_sb)
                nc.vector.tensor_copy(
                    cT_sb[hp * 64:(hp + 1) * 64, o, :],
                    trp[0:64, :, :].rearrange("p a b -> p (a b)"))

            def phase2_steps(o, pt):
                """Thunks: 8 ctx units + 2 transposes for pair-stage o."""
                c_sb = stg.tile([128, 2, SO, 64], bf16, name=f"c{o}", tag="c",
                                bufs=2)
                for hp in range(2):
                    for lt in range(SO):
                        yield lambda hp=hp, lt=lt: phase2_unit(
                            o, pt, hp, lt, c_sb)
                    yield lambda hp=hp: transpose_pair(o, hp, c_sb)

            # ---- 4 head-pair stages ----
            prev_p2 = None   # phase2 step iterator of the previous stage
            for o in range(SO):
                sched = mk_sched(o)
                pt = ptp.tile([128, NT, 1024], bf16, name=f"pt{o}",
                              tag="pt", bufs=2)
                for t in range(NT):
                    # fused score tile: head 2o in bank cols 0:512, head
                    # 2o+1 in 512:1024 (each matmul stays within one bank)
                    st2 = ps.tile([128, 1024], f32, name=f"st{o}_{t}",
                                  tag="st", bufs=2)
                    tsl = slice(t * 128, (t + 1) * 128)
                    nc.tensor.matmul(st2[:, 0:512], KT_sb[0:64, o, tsl],
                                     QT_sb[0:64, o, :], start=True, stop=True)
                    nc.tensor.matmul(st2[:, 512:1024], KT_sb[64:128, o, tsl],
                                     QT_sb[64:128, o, :], start=True, stop=True)
                    nc.scalar.activation(pt[:, t, :], st2, EXP)
                    # one phase2 step of the previous stage every other t
                    if t % 2 == 1 and prev_p2 is not None:
                        step = next(prev_p2, None)
                        if step is not None:
                            step()
                        if t == NT - 1:  # 10 steps total, drain leftovers
                            for step in prev_p2:
                                step()
                    for thunk in sched[t]:
                        thunk()
                prev_p2 = phase2_steps(o, pt)
            for step in prev_p2:
                step()

            # ---- out projection: out[lq, d] += cT[:, o, lq-sl].T @ wo ----
            # One staging tile + one DMA per lq tile (per-DMA fixed costs
            # dominate the tail otherwise).
            for lt in range(SO):
                lsl = slice(lt * 128, (lt + 1) * 128)
                ot = ost.tile([128, D], f32, name="ot", tag="ot")
                for dc in range(2):
                    dsl = slice(dc * 512, (dc + 1) * 512)
                    ops = ps.tile([128, 512], f32, name=f"ops{lt}_{dc}",
                                  tag="proj", bufs=2)
                    for o in range(SO):
                        nc.tensor.matmul(ops, cT_sb[:, o, lsl],
                                         wo_sb[:, o, dsl],
                                         start=(o == 0), stop=(o == SO - 1))
                    nc.scalar.activation(ot[:, dsl], ops, IDENT)
                nc.sync.dma_start(out[lt, :, :], ot)

    return _split_multi_waits(nc)


_NC = None


def _get_nc():
    global _NC
    if _NC is None:
        _NC = _build()
    return _NC


def _shard(q, kv, Wq, bq, Wk, bk, Wv, bv, Wo, bo):
    b16 = ml_dtypes.bfloat16

    def lay(a2d, co):  # [co*128, F] -> [128, co, F]
        F = a2d.shape[1]
        return np.ascontiguousarray(
            a2d.reshape(co, 128, F).transpose(1, 0, 2)).astype(b16)

    idn = np.eye(128, dtype=b16)
    in_maps = []
    for core in range(8):
        b, g = core // 2, core % 2
        sl = slice(g * S, (g + 1) * S)
        m = {
            "qT": lay(np.ascontiguousarray(q[b].T), CO),
            "kvT": lay(np.ascontiguousarray(kv[b].T), CO),
            "wqT": lay(np.ascontiguousarray((Wq[sl] * 0.125).T), CO),
            "wkT": lay(np.ascontiguousarray(Wk[sl].T), CO),
            "wvT": lay(np.ascontiguousarray(Wv[sl].T), CO),
            "woT": lay(np.ascontiguousarray(Wo[:, sl].T), SO),
            "bq": np.ascontiguousarray(
                (bq[sl] * 0.125).reshape(SO, 128).T).astype(np.float32),
            "bk": np.ascontiguousarray(
                bk[sl].reshape(SO, 128).T).astype(np.float32),
            "bvbc": np.broadcast_to(
                bv[sl].astype(b16), (128, S)).copy(),
            "ident": idn,
        }
        in_maps.append(m)
    return in_maps


def _run(in_maps, trace=False):
    res = run_bass_kernel_spmd(_get_nc(), in_maps, core_ids=list(range(8)),
                               trace=trace)
    return res


def kernel(q, kv, Wq, bq, Wk, bk, Wv, bv, Wo, bo, _trace=False):
    q, kv = np.asarray(q, np.float32), np.asarray(kv, np.float32)
    Wq, Wk = np.asarray(Wq, np.float32), np.asarray(Wk, np.float32)
    Wv, Wo = np.asarray(Wv, np.float32), np.asarray(Wo, np.float32)
    bq, bk = np.asarray(bq, np.float32), np.asarray(bk, np.float32)
    bv, bo = np.asarray(bv, np.float32), np.asarray(bo, np.float32)

    in_maps = _shard(q, kv, Wq, bq, Wk, bk, Wv, bv, Wo, bo)
    res = _run(in_maps, trace=_trace)
    B = q.shape[0]
    outp = np.empty((B, LQ, D), np.float32)
    for b in range(B):
        p0 = res.results[2 * b]["out"].reshape(LQ, D)
        p1 = res.results[2 * b + 1]["out"].reshape(LQ, D)
        outp[b] = p0 + p1 + bo[None, :]
    if _trace:
        kernel._last_exec_ns = res.exec_time_ns
        kernel._last_trace = res.instructions_and_trace
    return outp


# revision 14
# speedup vs baseline: 1.8733x; 1.0126x over previous
"""Cross-attention kernel for TRN2, 8 NeuronCores.

Sharding: core c -> (batch b = c//2, head-group g = c%2).  Each head-group is
8 heads = 512 of the 1024 d_model channels.  Within a core everything runs in
one fused software pipeline over 4 head-pair stages (o = 0..3):

  QT = wq_g.T @ q.T  (scale folded)       [512, 512]   (s, lq)
  KT = wk_g.T @ kv.T                      [512, 2048]  (s, lkv)
  V  = kv @ wv_g.T                        [2048, 512]  (lkv, s)   + ones col
  phase1(o), t = 0..15:
               ST = Kh.T-contract @ QT    [128, 512]   (lkv-tile, lq)
               P[t] = exp(ST)          -> bf16 SBUF [128, 16, 512] per head
  phase2(o), unit (hp, lt):  16 consecutive matmuls in ONE psum bank
               ctx[lq, 65] += P[t]_lt.T @ [Vh | 1]     (F=65 transposed form;
                                                        col 64 = softmax denom)
               C = ctx[:, 0:64] * recip(ctx[:, 64])  (DVE per-partition scalar)
  transpose C -> cT[s, lq]  (PE transpose, identity trick)
  out = cT.T @ wo_g.T                     [512, 1024]
Host sums the two head-group partials per batch and adds bo.

All operands are bf16 (1 cyc/row on PE at any free size; halves DMA), psum
accumulation f32.  phase2(o-1) and the projection matmuls of stage o+1 are
hand-interleaved into phase1(o)'s t-loop so the Act engine's exp stream (the
second largest engine load) fully overlaps PE work.  A psum accumulation
group owns its whole 2KB bank (start zeroes the full zero-region), hence the
consecutive-16 structure of phase2 rather than round-robin accumulation.
"""

import sys
if "/opt/trn_rl_repo" not in sys.path:
    sys.path.insert(0, "/opt/trn_rl_repo")

import numpy as np
import ml_dtypes

import concourse.bass as bass
import concourse.mybir as mybir
import concourse.tile as tile
from concourse.bass_utils import run_bass_kernel_spmd

f32 = mybir.dt.float32
bf16 = mybir.dt.bfloat16
EXP = mybir.ActivationFunctionType.Exp
IDENT = mybir.ActivationFunctionType.Identity

D = 1024        # d_model
S = 512         # per-core channel shard (8 heads x 64)
LQ = 512
LKV = 2048
CO = D // 128   # 8 contraction chunks
SO = S // 128   # 4 shard s-tiles (head pairs)
NT = LKV // 128  # 16 lkv tiles


def _split_multi_waits(nc, max_waits=1):
    """This container's walrus allows only `max_waits` sync-wait commands per
    instruction; hoist the excess into standalone EventSemaphore insts."""
    ev_id = 0
    for f in nc.m.functions:
        for bb in f.blocks:
            new = []
            changed = False
            for inst in bb.instructions:
                si = inst.sync_info
                if si is not None and si.on_wait and len(si.on_wait) > max_waits:
                    waits = list(si.on_wait)
                    for sw in waits[:-max_waits]:
                        ev = mybir.InstEventSemaphore(
                            name=f"EVSPLIT-{ev_id}", engine=inst.engine,
                            sync_info=mybir.SyncInfo(on_wait=[sw], on_update=[]))
                        ev_id += 1
                        nc.register_instruction(ev, overwrite=True)
                        new.append(ev)
                    inst.sync_info = mybir.SyncInfo(
                        on_wait=waits[-max_waits:], on_update=list(si.on_update))
                    changed = True
                new.append(inst)
            if changed:
                bb.instructions = new
    return nc


def _build():
    nc = bass.Bass(trn_type="TRN2")

    # DRAM I/O (pre-laid-out [128, outer, free] on host, bf16)
    qT = nc.dram_tensor("qT", [128, CO, LQ], bf16, kind="ExternalInput")
    kvT = nc.dram_tensor("kvT", [128, CO, LKV], bf16, kind="ExternalInput")
    wqT = nc.dram_tensor("wqT", [128, CO, S], bf16, kind="ExternalInput")
    wkT = nc.dram_tensor("wkT", [128, CO, S], bf16, kind="ExternalInput")
    wvT = nc.dram_tensor("wvT", [128, CO, S], bf16, kind="ExternalInput")
    woT = nc.dram_tensor("woT", [128, SO, D], bf16, kind="ExternalInput")
    bq = nc.dram_tensor("bq", [128, SO], f32, kind="ExternalInput")
    bk = nc.dram_tensor("bk", [128, SO], f32, kind="ExternalInput")
    bvbc = nc.dram_tensor("bvbc", [128, S], bf16, kind="ExternalInput")
    ident = nc.dram_tensor("ident", [128, 128], bf16, kind="ExternalInput")
    out = nc.dram_tensor("out", [SO, 128, D], f32, kind="ExternalOutput")

    with tile.TileContext(nc) as tc:
        with tc.tile_pool(name="wgt", bufs=1) as wgt, \
             tc.tile_pool(name="pt", bufs=2) as ptp, \
             tc.tile_pool(name="stg", bufs=4) as stg, \
             tc.tile_pool(name="ost", bufs=3) as ost, \
             tc.tile_pool(name="ps", bufs=1, space="PSUM") as ps:

            # ---- resident SBUF ----
            kv_sb = wgt.tile([128, CO, LKV], bf16, name="kv_sb")
            wk_sb = wgt.tile([128, CO, S], bf16, name="wk_sb")
            wv_sb = wgt.tile([128, CO, S], bf16, name="wv_sb")
            wq_sb = wgt.tile([128, CO, S], bf16, name="wq_sb")
            wo_sb = wgt.tile([128, SO, D], bf16, name="wo_sb")
            qT_sb = wgt.tile([128, CO, LQ], bf16, name="qT_sb")
            QT_sb = wgt.tile([128, SO, LQ], bf16, name="QT_sb")
            KT_sb = wgt.tile([128, SO, LKV], bf16, name="KT_sb")
            # V per head with a ones column: attn@V (transposed form) then
            # also yields the softmax denominator in output col 64.
            Vp_sb = wgt.tile([128, NT, 8, 65], bf16, name="Vp_sb")
            cT_sb = wgt.tile([128, SO, LQ], bf16, name="cT_sb")
            bq_sb = wgt.tile([128, SO], f32, name="bq_sb")
            bk_sb = wgt.tile([128, SO], f32, name="bk_sb")
            bvbc_sb = wgt.tile([128, S], bf16, name="bvbc_sb")
            ident_sb = wgt.tile([128, 128], bf16, name="ident_sb")

            # ---- DMA order = priority order (SP queue is serial).
            # Stage 0 only needs the o=0 slices of wq/wk, then the kv chunks
            # pace the stage-0 score loop; everything else arrives later.
            # Small tensors go between the chunks they are first needed after
            # (each DMA costs ~650ns of DGE issue latency regardless of size).
            nc.sync.dma_start(wk_sb[:, :, 0:128], wkT[:, :, 0:128])
            nc.sync.dma_start(wq_sb[:, :, 0:128], wqT[:, :, 0:128])
            nc.sync.dma_start(qT_sb, qT[:])
            nc.sync.dma_start(bq_sb, bq[:])
            nc.sync.dma_start(kv_sb[:, 0:4, 0:512], kvT[:, 0:4, 0:512])
            nc.sync.dma_start(kv_sb[:, 4:8, 0:512], kvT[:, 4:8, 0:512])
            nc.sync.dma_start(bk_sb, bk[:])
            for hh in range(2, 8):
                nc.sync.dma_start(kv_sb[:, 4 * (hh % 2):4 * (hh % 2) + 4,
                                        (hh // 2) * 512:(hh // 2 + 1) * 512],
                                  kvT[:, 4 * (hh % 2):4 * (hh % 2) + 4,
                                      (hh // 2) * 512:(hh // 2 + 1) * 512])
            nc.sync.dma_start(wv_sb, wvT[:])
            nc.sync.dma_start(bvbc_sb, bvbc[:])
            nc.sync.dma_start(ident_sb, ident[:])
            nc.sync.dma_start(wk_sb[:, :, 128:512], wkT[:, :, 128:512])
            nc.sync.dma_start(wq_sb[:, :, 128:512], wqT[:, :, 128:512])
            nc.sync.dma_start(wo_sb, woT[:])

            nc.vector.memset(Vp_sb[:, :, :, 64:65], 1.0)

            # ---- PE warm-up: the tensor engine runs at half clock until it
            # has been continuously busy for 3us.  Burn the initial DMA wait
            # on dummy matmuls so the real projections start at full speed.
            dm_sb = wgt.tile([128, 512], bf16, name="dm_sb")
            nc.vector.memset(dm_sb, 0.0)
            for i in range(24):
                dps = ps.tile([128, 512], f32, name=f"dps{i}", tag="proj",
                              bufs=2)
                nc.tensor.matmul(dps, dm_sb[:, 0:128], dm_sb,
                                 start=True, stop=True)

            # ---- emission helpers (each emits PE matmuls + its drain) ----
            def kproj(o, ch):
                kps = ps.tile([128, 512], f32, name=f"kps{o}_{ch}", tag="proj",
                              bufs=2)
                sl = slice(ch * 512, (ch + 1) * 512)
                for c in range(CO):
                    nc.tensor.matmul(kps, wk_sb[:, c, o * 128:(o + 1) * 128],
                                     kv_sb[:, c, sl],
                                     start=(c == 0), stop=(c == CO - 1))
                nc.vector.tensor_scalar_add(KT_sb[:, o, sl], kps,
                                            bk_sb[:, o:o + 1])

            def qproj(o):
                qps = ps.tile([128, 512], f32, name=f"qps{o}", tag="proj",
                              bufs=2)
                for c in range(CO):
                    nc.tensor.matmul(qps, wq_sb[:, c, o * 128:(o + 1) * 128],
                                     qT_sb[:, c, :],
                                     start=(c == 0), stop=(c == CO - 1))
                nc.vector.tensor_scalar_add(QT_sb[:, o, :], qps,
                                            bq_sb[:, o:o + 1])

            def vproj(o, t):
                vps = ps.tile([128, 128], f32, name=f"vps{o}_{t}", tag="proj",
                              bufs=2)
                tsl = slice(t * 128, (t + 1) * 128)
                osl = slice(o * 128, (o + 1) * 128)
                for c in range(CO):
                    nc.tensor.matmul(vps, kv_sb[:, c, tsl], wv_sb[:, c, osl],
                                     start=(c == 0), stop=(c == CO - 1))
                nc.vector.tensor_add(
                    Vp_sb[:, t, 2 * o:2 * o + 2, 0:64],
                    vps.rearrange("p (h d) -> p h d", h=2),
                    bvbc_sb[:, osl].rearrange("p (h d) -> p h d", h=2))

            # ---- lead-in: stage-0 prerequisites ----
            qproj(0)
            kproj(0, 0)

            # Per-stage fill schedules: iteration t -> thunks.  Placement
            # matches DMA arrival order (PE is in-order, so emitting a matmul
            # whose DMA lands late would stall everything behind it).
            def mk_sched(o):
                s = {t: [] for t in range(NT)}
                if o == 0:
                    # kv chunks land one per ~3.2us; kproj(0,ch) feeds the
                    # scores at t=4ch.  wv lands after kv3.
                    s[0].append(lambda: kproj(0, 1))
                    s[4].append(lambda: kproj(0, 2))
                    s[8].append(lambda: kproj(0, 3))
                    for t in range(8, NT):
                        s[t].append(lambda t=t: vproj(0, 2 * (t - 8)))
                        s[t].append(lambda t=t: vproj(0, 2 * (t - 8) + 1))
                else:
                    # own K chunks 1..3 first (ch0/qproj ran at the tail of
                    # the previous stage), V tiles just-in-time for phase2.
                    for ch in range(1, 4):
                        s[ch - 1].append(lambda ch=ch: kproj(o, ch))
                    for t in range(NT):
                        s[t].append(lambda t=t: vproj(o, t))
                if o < 3:
                    # next stage's Q and first K chunk at the stage tail
                    s[NT - 2].append(lambda: qproj(o + 1))
                    s[NT - 1].append(lambda: kproj(o + 1, 0))
                return s

            def phase2_unit(o, pt, hp, lt, c_sb):
                """ctx unit (head hp of pair o, lq tile lt): 16 consecutive
                matmuls in one psum bank, then normalize straight from psum.
                (An accumulation group owns its whole 2KB zero-region, so the
                16 steps must be consecutive in one dedicated bank.)
                Pair 3 runs at the kernel tail where Act is idle, so its
                normalize goes to the scalar engine instead of DVE, and its
                units alternate over the then-idle proj banks as well to keep
                4 accumulations in flight instead of 2."""
                if o == SO - 1 and (hp * SO + lt) % 2 == 1:
                    ctx = ps.tile([128, 65], f32, name=f"ctx{o}_{hp}_{lt}",
                                  tag="proj", bufs=2)
                else:
                    ctx = ps.tile([128, 65], f32, name=f"ctx{o}_{hp}_{lt}",
                                  tag="ctx", bufs=2)
                base = hp * 512 + lt * 128
                for t in range(NT):
                    nc.tensor.matmul(
                        ctx, pt[:, t, base:base + 128],
                        Vp_sb[:, t, 2 * o + hp, :],
                        start=(t == 0), stop=(t == NT - 1))
                rc = stg.tile([128, 1], f32, name=f"rc{o}_{hp}_{lt}", tag="rc",
                              bufs=4)
                nc.vector.reciprocal(rc, ctx[:, 64:65])
                if o == SO - 1:
                    nc.scalar.activation(c_sb[:, hp, lt, :], ctx[:, 0:64],
                                         IDENT, scale=rc)
                else:
                    nc.vector.tensor_scalar_mul(
                        c_sb[:, hp, lt, :], ctx[:, 0:64], rc)

            def transpose_pair(o, hp, c_sb):
                trp = ps.tile([128, SO, 128], bf16, name=f"trp{o}_{hp}",
                              tag="proj", bufs=2)
                for lt in range(SO):
                    nc.tensor.transpose(trp[0:64, lt, :],
                                        c_sb[:, hp, lt, :], ident_sb)
                nc.vector.tensor_copy(
                    cT_sb[hp * 64:(hp + 1) * 64, o, :],
                    trp[0:64, :, :].rearrange("p a b -> p (a b)"))

            def phase2_steps(o, pt):
                """Thunks: 8 ctx units + 2 transposes for pair-stage o."""
                c_sb = stg.tile([128, 2, SO, 64], bf16, name=f"c{o}", tag="c",
                                bufs=2)
                for hp in range(2):
                    for lt in range(SO):
                        yield lambda hp=hp, lt=lt: phase2_unit(
                            o, pt, hp, lt, c_sb)
                    yield lambda hp=hp: transpose_pair(o, hp, c_sb)

            # ---- 4 head-pair stages ----
            prev_p2 = None   # phase2 step iterator of the previous stage
            for o in range(SO):
                sched = mk_sched(o)
                pt = ptp.tile([128, NT, 1024], bf16, name=f"pt{o}",
                              tag="pt", bufs=2)
                for t in range(NT):
                    # fused score tile: head 2o in bank cols 0:512, head
                    # 2o+1 in 512:1024 (each matmul stays within one bank)
                    st2 = ps.tile([128, 1024], f32, name=f"st{o}_{t}",
                                  tag="st", bufs=2)
                    tsl = slice(t * 128, (t + 1) * 128)
                    nc.tensor.matmul(st2[:, 0:512], KT_sb[0:64, o, tsl],
                                     QT_sb[0:64, o, :], start=True, stop=True)
                    nc.tensor.matmul(st2[:, 512:1024], KT_sb[64:128, o, tsl],
                                     QT_sb[64:128, o, :], start=True, stop=True)
                    nc.scalar.activation(pt[:, t, :], st2, EXP)
                    # one phase2 step of the previous stage every other t
                    if t % 2 == 1 and prev_p2 is not None:
                        step = next(prev_p2, None)
                        if step is not None:
                            step()
                        if t == NT - 1:  # 10 steps total, drain leftovers
                            for step in prev_p2:
                                step()
                    for thunk in sched[t]:
                        thunk()
                prev_p2 = phase2_steps(o, pt)
            for step in prev_p2:
                step()

            # ---- out projection: out[lq, d] += cT[:, o, lq-sl].T @ wo ----
            # One staging tile + one DMA per lq tile (per-DMA fixed costs
            # dominate the tail otherwise).
            for lt in range(SO):
                lsl = slice(lt * 128, (lt + 1) * 128)
                ot = ost.tile([128, D], f32, name="ot", tag="ot")
                for dc in range(2):
                    dsl = slice(dc * 512, (dc + 1) * 512)
                    ops = ps.tile([128, 512], f32, name=f"ops{lt}_{dc}",
                                  tag="proj", bufs=2)
                    for o in range(SO):
                        nc.tensor.matmul(ops, cT_sb[:, o, lsl],
                                         wo_sb[:, o, dsl],
                                         start=(o == 0), stop=(o == SO - 1))
                    nc.scalar.activation(ot[:, dsl], ops, IDENT)
                nc.sync.dma_start(out[lt, :, :], ot)

    return _split_multi_waits(nc)


_NC = None


def _get_nc():
    global _NC
    if _NC is None:
        _NC = _build()
    return _NC


def _shard(q, kv, Wq, bq, Wk, bk, Wv, bv, Wo, bo):
    b16 = ml_dtypes.bfloat16

    def lay(a2d, co):  # [co*128, F] -> [128, co, F]
        F = a2d.shape[1]
        return np.ascontiguousarray(
            a2d.reshape(co, 128, F).transpose(1, 0, 2)).astype(b16)

    idn = np.eye(128, dtype=b16)
    in_maps = []
    for core in range(8):
        b, g = core // 2, core % 2
        sl = slice(g * S, (g + 1) * S)
        m = {
            "qT": lay(np.ascontiguousarray(q[b].T), CO),
            "kvT": lay(np.ascontiguousarray(kv[b].T), CO),
            "wqT": lay(np.ascontiguousarray((Wq[sl] * 0.125).T), CO),
            "wkT": lay(np.ascontiguousarray(Wk[sl].T), CO),
            "wvT": lay(np.ascontiguousarray(Wv[sl].T), CO),
            "woT": lay(np.ascontiguousarray(Wo[:, sl].T), SO),
            "bq": np.ascontiguousarray(
                (bq[sl] * 0.125).reshape(SO, 128).T).astype(np.float32),
            "bk": np.ascontiguousarray(
                bk[sl].reshape(SO, 128).T).astype(np.float32),
            "bvbc": np.broadcast_to(
                bv[sl].astype(b16), (128, S)).copy(),
            "ident": idn,
        }
        in_maps.append(m)
    return in_maps


def _run(in_maps, trace=False):
    res = run_bass_kernel_spmd(_get_nc(), in_maps, core_ids=list(range(8)),
                               trace=trace)
    return res


def kernel(q, kv, Wq, bq, Wk, bk, Wv, bv, Wo, bo, _trace=False):
    q, kv = np.asarray(q, np.float32), np.asarray(kv, np.float32)
    Wq, Wk = np.asarray(Wq, np.float32), np.asarray(Wk, np.float32)
    Wv, Wo = np.asarray(Wv, np.float32), np.asarray(Wo, np.float32)
    bq, bk = np.asarray(bq, np.float32), np.asarray(bk, np.float32)
    bv, bo = np.asarray(bv, np.float32), np.asarray(bo, np.float32)

    in_maps = _shard(q, kv, Wq, bq, Wk, bk, Wv, bv, Wo, bo)
    res = _run(in_maps, trace=_trace)
    B = q.shape[0]
    outp = np.empty((B, LQ, D), np.float32)
    for b in range(B):
        p0 = res.results[2 * b]["out"].reshape(LQ, D)
        p1 = res.results[2 * b + 1]["out"].reshape(LQ, D)
        outp[b] = p0 + p1 + bo[None, :]
    if _trace:
        kernel._last_exec_ns = res.exec_time_ns
        kernel._last_trace = res.instructions_and_trace
    return outp


# revision 17
# speedup vs baseline: 1.8816x; 1.0044x over previous
"""Cross-attention kernel for TRN2, 8 NeuronCores.

Sharding: core c -> (batch b = c//2, head-group g = c%2).  Each head-group is
8 heads = 512 of the 1024 d_model channels.  Within a core everything runs in
one fused software pipeline over 4 head-pair stages (o = 0..3):

  QT = wq_g.T @ q.T  (scale folded)       [512, 512]   (s, lq)
  KT = wk_g.T @ kv.T                      [512, 2048]  (s, lkv)
  V  = kv @ wv_g.T                        [2048, 512]  (lkv, s)   + ones col
  phase1(o), t = 0..15:
               ST = Kh.T-contract @ QT    [128, 512]   (lkv-tile, lq)
               P[t] = exp(ST)          -> bf16 SBUF [128, 16, 512] per head
  phase2(o), unit (hp, lt):  16 consecutive matmuls in ONE psum bank
               ctx[lq, 65] += P[t]_lt.T @ [Vh | 1]     (F=65 transposed form;
                                                        col 64 = softmax denom)
               C = ctx[:, 0:64] * recip(ctx[:, 64])  (DVE per-partition scalar)
  transpose C -> cT[s, lq]  (PE transpose, identity trick)
  out = cT.T @ wo_g.T                     [512, 1024]
Host sums the two head-group partials per batch and adds bo.

All operands are bf16 (1 cyc/row on PE at any free size; halves DMA), psum
accumulation f32.  phase2(o-1) and the projection matmuls of stage o+1 are
hand-interleaved into phase1(o)'s t-loop so the Act engine's exp stream (the
second largest engine load) fully overlaps PE work.  A psum accumulation
group owns its whole 2KB bank (start zeroes the full zero-region), hence the
consecutive-16 structure of phase2 rather than round-robin accumulation.
"""

import sys
if "/opt/trn_rl_repo" not in sys.path:
    sys.path.insert(0, "/opt/trn_rl_repo")

import numpy as np
import ml_dtypes

import concourse.bass as bass
import concourse.mybir as mybir
import concourse.tile as tile
from concourse.bass_utils import run_bass_kernel_spmd

f32 = mybir.dt.float32
bf16 = mybir.dt.bfloat16
EXP = mybir.ActivationFunctionType.Exp
IDENT = mybir.ActivationFunctionType.Identity

D = 1024        # d_model
S = 512         # per-core channel shard (8 heads x 64)
LQ = 512
LKV = 2048
CO = D // 128   # 8 contraction chunks
SO = S // 128   # 4 shard s-tiles (head pairs)
NT = LKV // 128  # 16 lkv tiles


def _split_multi_waits(nc, max_waits=1):
    """This container's walrus allows only `max_waits` sync-wait commands per
    instruction; hoist the excess into standalone EventSemaphore insts."""
    ev_id = 0
    for f in nc.m.functions:
        for bb in f.blocks:
            new = []
            changed = False
            for inst in bb.instructions:
                si = inst.sync_info
                if si is not None and si.on_wait and len(si.on_wait) > max_waits:
                    waits = list(si.on_wait)
                    for sw in waits[:-max_waits]:
                        ev = mybir.InstEventSemaphore(
                            name=f"EVSPLIT-{ev_id}", engine=inst.engine,
                            sync_info=mybir.SyncInfo(on_wait=[sw], on_update=[]))
                        ev_id += 1
                        nc.register_instruction(ev, overwrite=True)
                        new.append(ev)
                    inst.sync_info = mybir.SyncInfo(
                        on_wait=waits[-max_waits:], on_update=list(si.on_update))
                    changed = True
                new.append(inst)
            if changed:
                bb.instructions = new
    return nc


def _build():
    nc = bass.Bass(trn_type="TRN2")

    # DRAM I/O (pre-laid-out [128, outer, free] on host, bf16)
    qT = nc.dram_tensor("qT", [128, CO, LQ], bf16, kind="ExternalInput")
    kvT = nc.dram_tensor("kvT", [128, CO, LKV], bf16, kind="ExternalInput")
    wqT = nc.dram_tensor("wqT", [128, CO, S], bf16, kind="ExternalInput")
    wkT = nc.dram_tensor("wkT", [128, CO, S], bf16, kind="ExternalInput")
    wvT = nc.dram_tensor("wvT", [128, CO, S], bf16, kind="ExternalInput")
    woT = nc.dram_tensor("woT", [128, SO, D], bf16, kind="ExternalInput")
    bq = nc.dram_tensor("bq", [128, SO], f32, kind="ExternalInput")
    bk = nc.dram_tensor("bk", [128, SO], f32, kind="ExternalInput")
    bvbc = nc.dram_tensor("bvbc", [128, S], bf16, kind="ExternalInput")
    ident = nc.dram_tensor("ident", [128, 128], bf16, kind="ExternalInput")
    out = nc.dram_tensor("out", [SO, 128, D], bf16, kind="ExternalOutput")

    with tile.TileContext(nc) as tc:
        with tc.tile_pool(name="wgt", bufs=1) as wgt, \
             tc.tile_pool(name="pt", bufs=2) as ptp, \
             tc.tile_pool(name="stg", bufs=4) as stg, \
             tc.tile_pool(name="ost", bufs=3) as ost, \
             tc.tile_pool(name="ps", bufs=1, space="PSUM") as ps:

            # ---- resident SBUF ----
            kv_sb = wgt.tile([128, CO, LKV], bf16, name="kv_sb")
            wk_sb = wgt.tile([128, CO, S], bf16, name="wk_sb")
            wv_sb = wgt.tile([128, CO, S], bf16, name="wv_sb")
            wq_sb = wgt.tile([128, CO, S], bf16, name="wq_sb")
            wo_sb = wgt.tile([128, SO, D], bf16, name="wo_sb")
            qT_sb = wgt.tile([128, CO, LQ], bf16, name="qT_sb")
            QT_sb = wgt.tile([128, SO, LQ], bf16, name="QT_sb")
            KT_sb = wgt.tile([128, SO, LKV], bf16, name="KT_sb")
            # V per head with a ones column: attn@V (transposed form) then
            # also yields the softmax denominator in output col 64.
            Vp_sb = wgt.tile([128, NT, 8, 65], bf16, name="Vp_sb")
            cT_sb = wgt.tile([128, SO, LQ], bf16, name="cT_sb")
            bq_sb = wgt.tile([128, SO], f32, name="bq_sb")
            bk_sb = wgt.tile([128, SO], f32, name="bk_sb")
            bvbc_sb = wgt.tile([128, S], bf16, name="bvbc_sb")
            ident_sb = wgt.tile([128, 128], bf16, name="ident_sb")

            # ---- DMA order = priority order (SP queue is serial).
            # Stage 0 only needs the o=0 slices of wq/wk, then the kv chunks
            # pace the stage-0 score loop; everything else arrives later.
            # Small tensors go between the chunks they are first needed after
            # (each DMA costs ~650ns of DGE issue latency regardless of size).
            nc.sync.dma_start(wk_sb[:, :, 0:128], wkT[:, :, 0:128])
            nc.sync.dma_start(wq_sb[:, :, 0:128], wqT[:, :, 0:128])
            nc.sync.dma_start(qT_sb, qT[:])
            nc.sync.dma_start(bq_sb, bq[:])
            nc.sync.dma_start(kv_sb[:, 0:4, 0:512], kvT[:, 0:4, 0:512])
            nc.sync.dma_start(kv_sb[:, 4:8, 0:512], kvT[:, 4:8, 0:512])
            nc.sync.dma_start(bk_sb, bk[:])
            for hh in range(2, 8):
                nc.sync.dma_start(kv_sb[:, 4 * (hh % 2):4 * (hh % 2) + 4,
                                        (hh // 2) * 512:(hh // 2 + 1) * 512],
                                  kvT[:, 4 * (hh % 2):4 * (hh % 2) + 4,
                                      (hh // 2) * 512:(hh // 2 + 1) * 512])
            nc.sync.dma_start(wv_sb, wvT[:])
            nc.sync.dma_start(bvbc_sb, bvbc[:])
            nc.sync.dma_start(ident_sb, ident[:])
            nc.sync.dma_start(wk_sb[:, :, 128:512], wkT[:, :, 128:512])
            nc.sync.dma_start(wq_sb[:, :, 128:512], wqT[:, :, 128:512])
            nc.sync.dma_start(wo_sb, woT[:])

            nc.vector.memset(Vp_sb[:, :, :, 64:65], 1.0)

            # ---- PE warm-up: the tensor engine runs at half clock until it
            # has been continuously busy for 3us.  Burn the initial DMA wait
            # on dummy matmuls so the real projections start at full speed.
            dm_sb = wgt.tile([128, 512], bf16, name="dm_sb")
            nc.vector.memset(dm_sb, 0.0)
            for i in range(24):
                dps = ps.tile([128, 512], f32, name=f"dps{i}", tag="proj",
                              bufs=2)
                nc.tensor.matmul(dps, dm_sb[:, 0:128], dm_sb,
                                 start=True, stop=True)

            # ---- emission helpers (each emits PE matmuls + its drain) ----
            def kproj(o, ch):
                kps = ps.tile([128, 512], f32, name=f"kps{o}_{ch}", tag="proj",
                              bufs=2)
                sl = slice(ch * 512, (ch + 1) * 512)
                for c in range(CO):
                    nc.tensor.matmul(kps, wk_sb[:, c, o * 128:(o + 1) * 128],
                                     kv_sb[:, c, sl],
                                     start=(c == 0), stop=(c == CO - 1))
                nc.vector.tensor_scalar_add(KT_sb[:, o, sl], kps,
                                            bk_sb[:, o:o + 1])

            def qproj(o):
                qps = ps.tile([128, 512], f32, name=f"qps{o}", tag="proj",
                              bufs=2)
                for c in range(CO):
                    nc.tensor.matmul(qps, wq_sb[:, c, o * 128:(o + 1) * 128],
                                     qT_sb[:, c, :],
                                     start=(c == 0), stop=(c == CO - 1))
                nc.vector.tensor_scalar_add(QT_sb[:, o, :], qps,
                                            bq_sb[:, o:o + 1])

            def vproj(o, t):
                vps = ps.tile([128, 128], f32, name=f"vps{o}_{t}", tag="proj",
                              bufs=2)
                tsl = slice(t * 128, (t + 1) * 128)
                osl = slice(o * 128, (o + 1) * 128)
                for c in range(CO):
                    nc.tensor.matmul(vps, kv_sb[:, c, tsl], wv_sb[:, c, osl],
                                     start=(c == 0), stop=(c == CO - 1))
                nc.vector.tensor_add(
                    Vp_sb[:, t, 2 * o:2 * o + 2, 0:64],
                    vps.rearrange("p (h d) -> p h d", h=2),
                    bvbc_sb[:, osl].rearrange("p (h d) -> p h d", h=2))

            # ---- lead-in: stage-0 prerequisites ----
            qproj(0)
            kproj(0, 0)

            # Per-stage fill schedules: iteration t -> thunks.  Placement
            # matches DMA arrival order (PE is in-order, so emitting a matmul
            # whose DMA lands late would stall everything behind it).
            def mk_sched(o):
                s = {t: [] for t in range(NT)}
                if o == 0:
                    # kv chunks land one per ~3.2us; kproj(0,ch) feeds the
                    # scores at t=4ch.  wv lands after kv3.
                    s[0].append(lambda: kproj(0, 1))
                    s[4].append(lambda: kproj(0, 2))
                    s[8].append(lambda: kproj(0, 3))
                    for t in range(8, NT):
                        s[t].append(lambda t=t: vproj(0, 2 * (t - 8)))
                        s[t].append(lambda t=t: vproj(0, 2 * (t - 8) + 1))
                else:
                    # own K chunks 1..3 first (ch0/qproj ran at the tail of
                    # the previous stage), V tiles just-in-time for phase2.
                    for ch in range(1, 4):
                        s[ch - 1].append(lambda ch=ch: kproj(o, ch))
                    for t in range(NT):
                        s[t].append(lambda t=t: vproj(o, t))
                if o < 3:
                    # next stage's Q and first K chunk at the stage tail
                    s[NT - 2].append(lambda: qproj(o + 1))
                    s[NT - 1].append(lambda: kproj(o + 1, 0))
                return s

            def phase2_unit(o, pt, hp, lt, c_sb):
                """ctx unit (head hp of pair o, lq tile lt): 16 consecutive
                matmuls in one psum bank, then normalize straight from psum.
                (An accumulation group owns its whole 2KB zero-region, so the
                16 steps must be consecutive in one dedicated bank.)
                Pair 3 runs at the kernel tail where Act is idle, so its
                normalize goes to the scalar engine instead of DVE, and its
                units alternate over the then-idle proj banks as well to keep
                4 accumulations in flight instead of 2."""
                if o == SO - 1 and (hp * SO + lt) % 2 == 1:
                    ctx = ps.tile([128, 65], f32, name=f"ctx{o}_{hp}_{lt}",
                                  tag="proj", bufs=2)
                else:
                    ctx = ps.tile([128, 65], f32, name=f"ctx{o}_{hp}_{lt}",
                                  tag="ctx", bufs=2)
                base = hp * 512 + lt * 128
                for t in range(NT):
                    nc.tensor.matmul(
                        ctx, pt[:, t, base:base + 128],
                        Vp_sb[:, t, 2 * o + hp, :],
                        start=(t == 0), stop=(t == NT - 1))
                rc = stg.tile([128, 1], f32, name=f"rc{o}_{hp}_{lt}", tag="rc",
                              bufs=4)
                nc.vector.reciprocal(rc, ctx[:, 64:65])
                if o == SO - 1:
                    nc.scalar.activation(c_sb[:, hp, lt, :], ctx[:, 0:64],
                                         IDENT, scale=rc)
                else:
                    nc.vector.tensor_scalar_mul(
                        c_sb[:, hp, lt, :], ctx[:, 0:64], rc)

            def transpose_pair(o, hp, c_sb):
                trp = ps.tile([128, SO, 128], bf16, name=f"trp{o}_{hp}",
                              tag="proj", bufs=2)
                for lt in range(SO):
                    nc.tensor.transpose(trp[0:64, lt, :],
                                        c_sb[:, hp, lt, :], ident_sb)
                nc.vector.tensor_copy(
                    cT_sb[hp * 64:(hp + 1) * 64, o, :],
                    trp[0:64, :, :].rearrange("p a b -> p (a b)"))

            def phase2_steps(o, pt):
                """Thunks: 8 ctx units + 2 transposes for pair-stage o."""
                c_sb = stg.tile([128, 2, SO, 64], bf16, name=f"c{o}", tag="c",
                                bufs=2)
                for hp in range(2):
                    for lt in range(SO):
                        yield lambda hp=hp, lt=lt: phase2_unit(
                            o, pt, hp, lt, c_sb)
                    yield lambda hp=hp: transpose_pair(o, hp, c_sb)

            # ---- 4 head-pair stages ----
            prev_p2 = None   # phase2 step iterator of the previous stage
            for o in range(SO):
                sched = mk_sched(o)
                pt = ptp.tile([128, NT, 1024], bf16, name=f"pt{o}",
                              tag="pt", bufs=2)
                for t in range(NT):
                    # fused score tile: head 2o in bank cols 0:512, head
                    # 2o+1 in 512:1024 (each matmul stays within one bank)
                    st2 = ps.tile([128, 1024], f32, name=f"st{o}_{t}",
                                  tag="st", bufs=2)
                    tsl = slice(t * 128, (t + 1) * 128)
                    nc.tensor.matmul(st2[:, 0:512], KT_sb[0:64, o, tsl],
                                     QT_sb[0:64, o, :], start=True, stop=True)
                    nc.tensor.matmul(st2[:, 512:1024], KT_sb[64:128, o, tsl],
                                     QT_sb[64:128, o, :], start=True, stop=True)
                    nc.scalar.activation(pt[:, t, :], st2, EXP)
                    # one phase2 step of the previous stage every other t
                    if t % 2 == 1 and prev_p2 is not None:
                        step = next(prev_p2, None)
                        if step is not None:
                            step()
                        if t == NT - 1:  # 10 steps total, drain leftovers
                            for step in prev_p2:
                                step()
                    for thunk in sched[t]:
                        thunk()
                prev_p2 = phase2_steps(o, pt)
            for step in prev_p2:
                step()

            # ---- out projection: out[lq, d] += cT[:, o, lq-sl].T @ wo ----
            # One staging tile + one DMA per lq tile (per-DMA fixed costs
            # dominate the tail otherwise).
            for lt in range(SO):
                lsl = slice(lt * 128, (lt + 1) * 128)
                ot = ost.tile([128, D], bf16, name="ot", tag="ot")
                for dc in range(2):
                    dsl = slice(dc * 512, (dc + 1) * 512)
                    ops = ps.tile([128, 512], f32, name=f"ops{lt}_{dc}",
                                  tag="proj", bufs=2)
                    for o in range(SO):
                        nc.tensor.matmul(ops, cT_sb[:, o, lsl],
                                         wo_sb[:, o, dsl],
                                         start=(o == 0), stop=(o == SO - 1))
                    if dc == 0:
                        nc.scalar.activation(ot[:, dsl], ops, IDENT)
                    else:
                        nc.vector.tensor_copy(ot[:, dsl], ops)
                nc.sync.dma_start(out[lt, :, :], ot)

    return _split_multi_waits(nc)


_NC = None


def _get_nc():
    global _NC
    if _NC is None:
        _NC = _build()
    return _NC


def _shard(q, kv, Wq, bq, Wk, bk, Wv, bv, Wo, bo):
    b16 = ml_dtypes.bfloat16

    def lay(a2d, co):  # [co*128, F] -> [128, co, F]
        F = a2d.shape[1]
        return np.ascontiguousarray(
            a2d.reshape(co, 128, F).transpose(1, 0, 2)).astype(b16)

    idn = np.eye(128, dtype=b16)
    in_maps = []
    for core in range(8):
        b, g = core // 2, core % 2
        sl = slice(g * S, (g + 1) * S)
        m = {
            "qT": lay(np.ascontiguousarray(q[b].T), CO),
            "kvT": lay(np.ascontiguousarray(kv[b].T), CO),
            "wqT": lay(np.ascontiguousarray((Wq[sl] * 0.125).T), CO),
            "wkT": lay(np.ascontiguousarray(Wk[sl].T), CO),
            "wvT": lay(np.ascontiguousarray(Wv[sl].T), CO),
            "woT": lay(np.ascontiguousarray(Wo[:, sl].T), SO),
            "bq": np.ascontiguousarray(
                (bq[sl] * 0.125).reshape(SO, 128).T).astype(np.float32),
            "bk": np.ascontiguousarray(
                bk[sl].reshape(SO, 128).T).astype(np.float32),
            "bvbc": np.broadcast_to(
                bv[sl].astype(b16), (128, S)).copy(),
            "ident": idn,
        }
        in_maps.append(m)
    return in_maps


def _run(in_maps, trace=False):
    res = run_bass_kernel_spmd(_get_nc(), in_maps, core_ids=list(range(8)),
                               trace=trace)
    return res


def kernel(q, kv, Wq, bq, Wk, bk, Wv, bv, Wo, bo, _trace=False):
    q, kv = np.asarray(q, np.float32), np.asarray(kv, np.float32)
    Wq, Wk = np.asarray(Wq, np.float32), np.asarray(Wk, np.float32)
    Wv, Wo = np.asarray(Wv, np.float32), np.asarray(Wo, np.float32)
    bq, bk = np.asarray(bq, np.float32), np.asarray(bk, np.float32)
    bv, bo = np.asarray(bv, np.float32), np.asarray(bo, np.float32)

    in_maps = _shard(q, kv, Wq, bq, Wk, bk, Wv, bv, Wo, bo)
    res = _run(in_maps, trace=_trace)
    B = q.shape[0]
    outp = np.empty((B, LQ, D), np.float32)
    for b in range(B):
        p0 = np.asarray(res.results[2 * b]["out"],
                        np.float32).reshape(LQ, D)
        p1 = np.asarray(res.results[2 * b + 1]["out"],
                        np.float32).reshape(LQ, D)
        outp[b] = p0 + p1 + bo[None, :]
    if _trace:
        kernel._last_exec_ns = res.exec_time_ns
        kernel._last_trace = res.instructions_and_trace
    return outp


# revision 18
# speedup vs baseline: 1.8877x; 1.0032x over previous
"""Cross-attention kernel for TRN2, 8 NeuronCores.

Sharding: core c -> (batch b = c//2, head-group g = c%2).  Each head-group is
8 heads = 512 of the 1024 d_model channels.  Within a core everything runs in
one fused software pipeline over 4 head-pair stages (o = 0..3):

  QT = wq_g.T @ q.T  (scale folded)       [512, 512]   (s, lq)
  KT = wk_g.T @ kv.T                      [512, 2048]  (s, lkv)
  V  = kv @ wv_g.T                        [2048, 512]  (lkv, s)   + ones col
  phase1(o), t = 0..15:
               ST = Kh.T-contract @ QT    [128, 512]   (lkv-tile, lq)
               P[t] = exp(ST)          -> bf16 SBUF [128, 16, 512] per head
  phase2(o), unit (hp, lt):  16 consecutive matmuls in ONE psum bank
               ctx[lq, 65] += P[t]_lt.T @ [Vh | 1]     (F=65 transposed form;
                                                        col 64 = softmax denom)
               C = ctx[:, 0:64] * recip(ctx[:, 64])  (DVE per-partition scalar)
  transpose C -> cT[s, lq]  (PE transpose, identity trick)
  out = cT.T @ wo_g.T                     [512, 1024]
Host sums the two head-group partials per batch and adds bo.

All operands are bf16 (1 cyc/row on PE at any free size; halves DMA), psum
accumulation f32.  phase2(o-1) and the projection matmuls of stage o+1 are
hand-interleaved into phase1(o)'s t-loop so the Act engine's exp stream (the
second largest engine load) fully overlaps PE work.  A psum accumulation
group owns its whole 2KB bank (start zeroes the full zero-region), hence the
consecutive-16 structure of phase2 rather than round-robin accumulation.
"""

import sys
if "/opt/trn_rl_repo" not in sys.path:
    sys.path.insert(0, "/opt/trn_rl_repo")

import numpy as np
import ml_dtypes

import concourse.bass as bass
import concourse.mybir as mybir
import concourse.tile as tile
from concourse.bass_utils import run_bass_kernel_spmd

f32 = mybir.dt.float32
bf16 = mybir.dt.bfloat16
EXP = mybir.ActivationFunctionType.Exp
IDENT = mybir.ActivationFunctionType.Identity

D = 1024        # d_model
S = 512         # per-core channel shard (8 heads x 64)
LQ = 512
LKV = 2048
CO = D // 128   # 8 contraction chunks
SO = S // 128   # 4 shard s-tiles (head pairs)
NT = LKV // 128  # 16 lkv tiles


def _split_multi_waits(nc, max_waits=1):
    """This container's walrus allows only `max_waits` sync-wait commands per
    instruction; hoist the excess into standalone EventSemaphore insts."""
    ev_id = 0
    for f in nc.m.functions:
        for bb in f.blocks:
            new = []
            changed = False
            for inst in bb.instructions:
                si = inst.sync_info
                if si is not None and si.on_wait and len(si.on_wait) > max_waits:
                    waits = list(si.on_wait)
                    for sw in waits[:-max_waits]:
                        ev = mybir.InstEventSemaphore(
                            name=f"EVSPLIT-{ev_id}", engine=inst.engine,
                            sync_info=mybir.SyncInfo(on_wait=[sw], on_update=[]))
                        ev_id += 1
                        nc.register_instruction(ev, overwrite=True)
                        new.append(ev)
                    inst.sync_info = mybir.SyncInfo(
                        on_wait=waits[-max_waits:], on_update=list(si.on_update))
                    changed = True
                new.append(inst)
            if changed:
                bb.instructions = new
    return nc


def _build():
    nc = bass.Bass(trn_type="TRN2")

    # DRAM I/O (pre-laid-out [128, outer, free] on host, bf16)
    qT = nc.dram_tensor("qT", [128, CO, LQ], bf16, kind="ExternalInput")
    kvT = nc.dram_tensor("kvT", [128, CO, LKV], bf16, kind="ExternalInput")
    wqT = nc.dram_tensor("wqT", [128, CO, S], bf16, kind="ExternalInput")
    wkT = nc.dram_tensor("wkT", [128, CO, S], bf16, kind="ExternalInput")
    wvT = nc.dram_tensor("wvT", [128, CO, S], bf16, kind="ExternalInput")
    woT = nc.dram_tensor("woT", [128, SO, D], bf16, kind="ExternalInput")
    bq = nc.dram_tensor("bq", [128, SO], f32, kind="ExternalInput")
    bk = nc.dram_tensor("bk", [128, SO], f32, kind="ExternalInput")
    bvbc = nc.dram_tensor("bvbc", [128, S], bf16, kind="ExternalInput")
    ident = nc.dram_tensor("ident", [128, 128], bf16, kind="ExternalInput")
    out = nc.dram_tensor("out", [SO, 128, D], bf16, kind="ExternalOutput")

    with tile.TileContext(nc) as tc:
        with tc.tile_pool(name="wgt", bufs=1) as wgt, \
             tc.tile_pool(name="pt", bufs=2) as ptp, \
             tc.tile_pool(name="stg", bufs=4) as stg, \
             tc.tile_pool(name="ost", bufs=3) as ost, \
             tc.tile_pool(name="ps", bufs=1, space="PSUM") as ps:

            # ---- resident SBUF ----
            kv_sb = wgt.tile([128, CO, LKV], bf16, name="kv_sb")
            wk_sb = wgt.tile([128, CO, S], bf16, name="wk_sb")
            wv_sb = wgt.tile([128, CO, S], bf16, name="wv_sb")
            wq_sb = wgt.tile([128, CO, S], bf16, name="wq_sb")
            wo_sb = wgt.tile([128, SO, D], bf16, name="wo_sb")
            qT_sb = wgt.tile([128, CO, LQ], bf16, name="qT_sb")
            QT_sb = wgt.tile([128, SO, LQ], bf16, name="QT_sb")
            KT_sb = wgt.tile([128, SO, LKV], bf16, name="KT_sb")
            # V per head with a ones column: attn@V (transposed form) then
            # also yields the softmax denominator in output col 64.
            Vp_sb = wgt.tile([128, NT, 8, 65], bf16, name="Vp_sb")
            cT_sb = wgt.tile([128, SO, LQ], bf16, name="cT_sb")
            bq_sb = wgt.tile([128, SO], f32, name="bq_sb")
            bk_sb = wgt.tile([128, SO], f32, name="bk_sb")
            bvbc_sb = wgt.tile([128, S], bf16, name="bvbc_sb")
            ident_sb = wgt.tile([128, 128], bf16, name="ident_sb")

            # ---- DMA order = priority order (SP queue is serial).
            # Stage 0 only needs the o=0 slices of wq/wk, then the kv chunks
            # pace the stage-0 score loop; everything else arrives later.
            # Small tensors go between the chunks they are first needed after
            # (each DMA costs ~650ns of DGE issue latency regardless of size).
            nc.sync.dma_start(wk_sb[:, :, 0:128], wkT[:, :, 0:128])
            nc.sync.dma_start(wq_sb[:, :, 0:128], wqT[:, :, 0:128])
            nc.sync.dma_start(qT_sb, qT[:])
            nc.sync.dma_start(bq_sb, bq[:])
            nc.sync.dma_start(kv_sb[:, 0:4, 0:512], kvT[:, 0:4, 0:512])
            nc.sync.dma_start(kv_sb[:, 4:8, 0:512], kvT[:, 4:8, 0:512])
            nc.sync.dma_start(bk_sb, bk[:])
            for hh in range(2, 8):
                nc.sync.dma_start(kv_sb[:, 4 * (hh % 2):4 * (hh % 2) + 4,
                                        (hh // 2) * 512:(hh // 2 + 1) * 512],
                                  kvT[:, 4 * (hh % 2):4 * (hh % 2) + 4,
                                      (hh // 2) * 512:(hh // 2 + 1) * 512])
            nc.sync.dma_start(wv_sb, wvT[:])
            nc.sync.dma_start(bvbc_sb, bvbc[:])
            nc.sync.dma_start(ident_sb, ident[:])
            nc.sync.dma_start(wk_sb[:, :, 128:512], wkT[:, :, 128:512])
            nc.sync.dma_start(wq_sb[:, :, 128:512], wqT[:, :, 128:512])
            nc.sync.dma_start(wo_sb, woT[:])

            nc.vector.memset(Vp_sb[:, :, :, 64:65], 1.0)

            # ---- PE warm-up: the tensor engine runs at half clock until it
            # has been continuously busy for 3us.  Burn the initial DMA wait
            # on dummy matmuls so the real projections start at full speed.
            dm_sb = wgt.tile([128, 512], bf16, name="dm_sb")
            nc.vector.memset(dm_sb, 0.0)
            for i in range(24):
                dps = ps.tile([128, 512], f32, name=f"dps{i}", tag="proj",
                              bufs=2)
                nc.tensor.matmul(dps, dm_sb[:, 0:128], dm_sb,
                                 start=True, stop=True)

            # ---- emission helpers (each emits PE matmuls + its drain) ----
            def kproj(o, ch):
                kps = ps.tile([128, 512], f32, name=f"kps{o}_{ch}", tag="proj",
                              bufs=2)
                sl = slice(ch * 512, (ch + 1) * 512)
                for c in range(CO):
                    nc.tensor.matmul(kps, wk_sb[:, c, o * 128:(o + 1) * 128],
                                     kv_sb[:, c, sl],
                                     start=(c == 0), stop=(c == CO - 1))
                nc.vector.tensor_scalar_add(KT_sb[:, o, sl], kps,
                                            bk_sb[:, o:o + 1])

            def qproj(o):
                qps = ps.tile([128, 512], f32, name=f"qps{o}", tag="proj",
                              bufs=2)
                for c in range(CO):
                    nc.tensor.matmul(qps, wq_sb[:, c, o * 128:(o + 1) * 128],
                                     qT_sb[:, c, :],
                                     start=(c == 0), stop=(c == CO - 1))
                nc.vector.tensor_scalar_add(QT_sb[:, o, :], qps,
                                            bq_sb[:, o:o + 1])

            def vproj(o, t):
                vps = ps.tile([128, 128], f32, name=f"vps{o}_{t}", tag="proj",
                              bufs=2)
                tsl = slice(t * 128, (t + 1) * 128)
                osl = slice(o * 128, (o + 1) * 128)
                for c in range(CO):
                    nc.tensor.matmul(vps, kv_sb[:, c, tsl], wv_sb[:, c, osl],
                                     start=(c == 0), stop=(c == CO - 1))
                nc.vector.tensor_add(
                    Vp_sb[:, t, 2 * o:2 * o + 2, 0:64],
                    vps.rearrange("p (h d) -> p h d", h=2),
                    bvbc_sb[:, osl].rearrange("p (h d) -> p h d", h=2))

            # ---- lead-in: stage-0 prerequisites ----
            qproj(0)
            kproj(0, 0)

            # Per-stage fill schedules: iteration t -> thunks.  Placement
            # matches DMA arrival order (PE is in-order, so emitting a matmul
            # whose DMA lands late would stall everything behind it).
            def mk_sched(o):
                s = {t: [] for t in range(NT)}
                if o == 0:
                    # kv chunks land one per ~3.2us; kproj(0,ch) feeds the
                    # scores at t=4ch.  wv lands after kv3.
                    s[0].append(lambda: kproj(0, 1))
                    s[4].append(lambda: kproj(0, 2))
                    s[8].append(lambda: kproj(0, 3))
                    for t in range(8, NT):
                        s[t].append(lambda t=t: vproj(0, 2 * (t - 8)))
                        s[t].append(lambda t=t: vproj(0, 2 * (t - 8) + 1))
                else:
                    # own K chunks 1..3 first (ch0/qproj ran at the tail of
                    # the previous stage), V tiles just-in-time for phase2.
                    for ch in range(1, 4):
                        s[ch - 1].append(lambda ch=ch: kproj(o, ch))
                    for t in range(NT):
                        s[t].append(lambda t=t: vproj(o, t))
                if o < 3:
                    # next stage's Q and first K chunk at the stage tail
                    s[NT - 2].append(lambda: qproj(o + 1))
                    s[NT - 1].append(lambda: kproj(o + 1, 0))
                return s

            def phase2_unit(o, pt, hp, lt, c_sb):
                """ctx unit (head hp of pair o, lq tile lt): 16 consecutive
                matmuls in one psum bank, then normalize straight from psum.
                (An accumulation group owns its whole 2KB zero-region, so the
                16 steps must be consecutive in one dedicated bank.)
                Pair 3 runs at the kernel tail where Act is idle, so its
                normalize goes to the scalar engine instead of DVE, and its
                units alternate over the then-idle proj banks as well to keep
                4 accumulations in flight instead of 2."""
                if o == SO - 1 and (hp * SO + lt) % 2 == 1:
                    ctx = ps.tile([128, 65], f32, name=f"ctx{o}_{hp}_{lt}",
                                  tag="proj", bufs=2)
                else:
                    ctx = ps.tile([128, 65], f32, name=f"ctx{o}_{hp}_{lt}",
                                  tag="ctx", bufs=2)
                base = hp * 512 + lt * 128
                for t in range(NT):
                    nc.tensor.matmul(
                        ctx, pt[:, t, base:base + 128],
                        Vp_sb[:, t, 2 * o + hp, :],
                        start=(t == 0), stop=(t == NT - 1))
                rc = stg.tile([128, 1], f32, name=f"rc{o}_{hp}_{lt}", tag="rc",
                              bufs=4)
                nc.vector.reciprocal(rc, ctx[:, 64:65])
                if o == SO - 1:
                    nc.scalar.activation(c_sb[:, hp, lt, :], ctx[:, 0:64],
                                         IDENT, scale=rc)
                else:
                    nc.vector.tensor_scalar_mul(
                        c_sb[:, hp, lt, :], ctx[:, 0:64], rc)

            def transpose_pair(o, hp, c_sb):
                trp = ps.tile([128, SO, 128], bf16, name=f"trp{o}_{hp}",
                              tag="proj", bufs=2)
                for lt in range(SO):
                    nc.tensor.transpose(trp[0:64, lt, :],
                                        c_sb[:, hp, lt, :], ident_sb)
                nc.vector.tensor_copy(
                    cT_sb[hp * 64:(hp + 1) * 64, o, :],
                    trp[0:64, :, :].rearrange("p a b -> p (a b)"))

            def phase2_steps(o, pt):
                """Thunks: 8 ctx units + 2 transposes for pair-stage o."""
                c_sb = stg.tile([128, 2, SO, 64], bf16, name=f"c{o}", tag="c",
                                bufs=2)
                for hp in range(2):
                    for lt in range(SO):
                        yield lambda hp=hp, lt=lt: phase2_unit(
                            o, pt, hp, lt, c_sb)
                    yield lambda hp=hp: transpose_pair(o, hp, c_sb)

            # ---- 4 head-pair stages ----
            prev_p2 = None   # phase2 step iterator of the previous stage
            for o in range(SO):
                sched = mk_sched(o)
                pt = ptp.tile([128, NT, 1024], bf16, name=f"pt{o}",
                              tag="pt", bufs=2)
                for t in range(NT):
                    # fused score tile: head 2o in bank cols 0:512, head
                    # 2o+1 in 512:1024 (each matmul stays within one bank)
                    st2 = ps.tile([128, 1024], f32, name=f"st{o}_{t}",
                                  tag="st", bufs=2)
                    tsl = slice(t * 128, (t + 1) * 128)
                    nc.tensor.matmul(st2[:, 0:512], KT_sb[0:64, o, tsl],
                                     QT_sb[0:64, o, :], start=True, stop=True)
                    nc.tensor.matmul(st2[:, 512:1024], KT_sb[64:128, o, tsl],
                                     QT_sb[64:128, o, :], start=True, stop=True)
                    nc.scalar.activation(pt[:, t, :], st2, EXP)
                    # one phase2 step of the previous stage every other t
                    if t % 2 == 1 and prev_p2 is not None:
                        step = next(prev_p2, None)
                        if step is not None:
                            step()
                        if t == NT - 1:  # 10 steps total, drain leftovers
                            for step in prev_p2:
                                step()
                    for thunk in sched[t]:
                        thunk()
                prev_p2 = phase2_steps(o, pt)
            for step in prev_p2:
                step()

            # ---- out projection: out[lq, d] += cT[:, o, lq-sl].T @ wo ----
            # One staging tile + one DMA per lq tile (per-DMA fixed costs
            # dominate the tail otherwise).
            for lt in range(SO):
                lsl = slice(lt * 128, (lt + 1) * 128)
                ot = ost.tile([128, D], bf16, name="ot", tag="ot")
                for dc in range(2):
                    dsl = slice(dc * 512, (dc + 1) * 512)
                    ops = ps.tile([128, 512], f32, name=f"ops{lt}_{dc}",
                                  tag="proj", bufs=2)
                    for o in range(SO):
                        nc.tensor.matmul(ops, cT_sb[:, o, lsl],
                                         wo_sb[:, o, dsl],
                                         start=(o == 0), stop=(o == SO - 1))
                    if dc == 0:
                        nc.scalar.activation(ot[:, dsl], ops, IDENT)
                    else:
                        nc.vector.tensor_copy(ot[:, dsl], ops)
                    if lt == SO - 1:
                        # final tile: per-half DMAs so the kernel end only
                        # waits on the last half's drain
                        nc.sync.dma_start(out[lt, :, dsl], ot[:, dsl])
                if lt < SO - 1:
                    nc.sync.dma_start(out[lt, :, :], ot)

    return _split_multi_waits(nc)


_NC = None


def _get_nc():
    global _NC
    if _NC is None:
        _NC = _build()
    return _NC


def _shard(q, kv, Wq, bq, Wk, bk, Wv, bv, Wo, bo):
    b16 = ml_dtypes.bfloat16

    def lay(a2d, co):  # [co*128, F] -> [128, co, F]
        F = a2d.shape[1]
        return np.ascontiguousarray(
            a2d.reshape(co, 128, F).transpose(1, 0, 2)).astype(b16)

    idn = np.eye(128, dtype=b16)
    in_maps = []
    for core in range(8):
        b, g = core // 2, core % 2
        sl = slice(g * S, (g + 1) * S)
        m = {
            "qT": lay(np.ascontiguousarray(q[b].T), CO),
            "kvT": lay(np.ascontiguousarray(kv[b].T), CO),
            "wqT": lay(np.ascontiguousarray((Wq[sl] * 0.125).T), CO),
            "wkT": lay(np.ascontiguousarray(Wk[sl].T), CO),
            "wvT": lay(np.ascontiguousarray(Wv[sl].T), CO),
            "woT": lay(np.ascontiguousarray(Wo[:, sl].T), SO),
            "bq": np.ascontiguousarray(
                (bq[sl] * 0.125).reshape(SO, 128).T).astype(np.float32),
            "bk": np.ascontiguousarray(
                bk[sl].reshape(SO, 128).T).astype(np.float32),
            "bvbc": np.broadcast_to(
                bv[sl].astype(b16), (128, S)).copy(),
            "ident": idn,
        }
        in_maps.append(m)
    return in_maps


def _run(in_maps, trace=False):
    res = run_bass_kernel_spmd(_get_nc(), in_maps, core_ids=list(range(8)),
                               trace=trace)
    return res


def kernel(q, kv, Wq, bq, Wk, bk, Wv, bv, Wo, bo, _trace=False):
    q, kv = np.asarray(q, np.float32), np.asarray(kv, np.float32)
    Wq, Wk = np.asarray(Wq, np.float32), np.asarray(Wk, np.float32)
    Wv, Wo = np.asarray(Wv, np.float32), np.asarray(Wo, np.float32)
    bq, bk = np.asarray(bq, np.float32), np.asarray(bk, np.float32)
    bv, bo = np.asarray(bv, np.float32), np.asarray(bo, np.float32)

    in_maps = _shard(q, kv, Wq, bq, Wk, bk, Wv, bv, Wo, bo)
    res = _run(in_maps, trace=_trace)
    B = q.shape[0]
    outp = np.empty((B, LQ, D), np.float32)
    for b in range(B):
        p0 = np.asarray(res.results[2 * b]["out"],
                        np.float32).reshape(LQ, D)
        p1 = np.asarray(res.results[2 * b + 1]["out"],
                        np.float32).reshape(LQ, D)
        outp[b] = p0 + p1 + bo[None, :]
    if _trace:
        kernel._last_exec_ns = res.exec_time_ns
        kernel._last_trace = res.instructions_and_trace
    return outp
